# revision 1
# baseline (speedup 1.0000x reference)
"""Trainium2 Bass kernel for CrossViewDeformableBlock (sparse deformable attention).

Contract: kernel(**inputs) -> np.ndarray takes FULL inputs (as from
setup_inputs()) and returns the FULL output [b, 128, 64, 64].

Sharding: 8 cores, q-parallel. Core c handles batch b_c = c//4 and query
range [(c%4)*1024, +1024) of the 64*64=4096 BEV queries. Each core builds
the bf16 K|V image tables for its 6 cameras on-device, computes projection
/ offsets / bilinear sample coordinates on-device, gathers 2-position rows
with dma_gather, blends corners on DVE, does the point-softmax attention
and output projection, and writes its [1024, 128] output shard. The host
only slices inputs, transposes weights (layout), and concatenates shards.
"""

import math
import os
import numpy as np

import concourse.bass as bass
import concourse.mybir as mybir
import concourse.tile as tile
from concourse import bacc
from concourse.bass import ts
from concourse.masks import make_identity

# ---------------------------------------------------------------- constants
B, NCAM, H, W = 2, 6, 64, 64
HW = H * W                      # 4096 queries per batch
IH, IW = 32, 88                 # image feature h, w
IHW = IH * IW                   # 2816 positions
HEADS, DH, INNER = 4, 32, 128
NP = 8                          # sample points per query
DIM = 128
NCORES = 8
QPC = HW // (NCORES // B)       # 1024 queries per core
NQT = QPC // 128                # 8 q-tiles of 128
PADROWS = 2944                  # 23 * 128 rows in kv table (2816 + 128 pad)
KVROW = 2 * INNER               # 256 channels (K|V) per position
F32 = mybir.dt.float32
BF16 = mybir.dt.bfloat16
I16 = mybir.dt.int16
I32 = mybir.dt.int32

_USE_CUSTOM_LERP = True
DEBUG = False


def _register_lerp_op():
    """Register LERP2: out = in0*s0 + in1*s1 (per-partition scalars s0,s1)."""
    from concourse.dve_spec import Spec, Src0, Src1, C0, C1, lower
    from concourse.dve_spec import _has_src1 as has_src1
    from concourse.dve_uop import DveOpSpec
    from concourse.dve_ops import DveOp, OPS, _SUB_OPCODE_FOR_NAME, _CUSTOM_DVE_ROW_BASE

    if "LERP2" in _SUB_OPCODE_FOR_NAME:
        for op in OPS:
            if op.name == "LERP2":
                return op
    spec = Spec(
        body=Src0 * C0 + Src1 * C1,
        reference=lambda in0, in1, s0, s1, imm2: (
            in0.astype(np.float32) * s0 + in1.astype(np.float32) * s1
        ),
    )
    opcode = _CUSTOM_DVE_ROW_BASE + len(OPS)
    assert opcode < 0x20
    shas = {}
    for ver in ("v3", "v4"):
        try:
            r = DveOpSpec(name="LERP2", opcode=opcode, uops=lower(spec, ver=ver),
                          rd1_en=has_src1(spec))
            shas[ver] = r.sha(ver)
        except Exception:
            pass
    op = DveOp("LERP2", spec, subdim=False, uops_sha=shas,
               perf_en={v: True for v in shas})
    OPS.append(op)
    _SUB_OPCODE_FOR_NAME["LERP2"] = opcode
    from concourse import dve_ops as _do
    _do.CUSTOM_DVE_SPECS["LERP2"] = spec
    return op


def build_kernel(nc):
    """Emit the SPMD program. All per-core variation comes via input data."""
    lerp_op = _register_lerp_op() if _USE_CUSTOM_LERP else None

    # ---------------- dram I/O ----------------
    img = nc.dram_tensor("img", [NCAM, DIM, IHW], F32, kind="ExternalInput").ap()
    wkvT = nc.dram_tensor("wkvT", [DIM, KVROW], F32, kind="ExternalInput").ap()
    bev_l = nc.dram_tensor("bev_l", [DIM, QPC], F32, kind="ExternalInput").ap()
    wxy_l = nc.dram_tensor("wxy_l", [2, QPC], F32, kind="ExternalInput").ap()
    ET = nc.dram_tensor("ET", [4, NCAM * 4], F32, kind="ExternalInput").ap()
    KT = nc.dram_tensor("KT", [3, NCAM * 3], F32, kind="ExternalInput").ap()
    wqT = nc.dram_tensor("wqT", [DIM, INNER], F32, kind="ExternalInput").ap()
    bq = nc.dram_tensor("bq", [1, INNER], F32, kind="ExternalInput").ap()
    w1T = nc.dram_tensor("w1T", [DIM, DIM], F32, kind="ExternalInput").ap()
    b1 = nc.dram_tensor("b1", [DIM, 1], F32, kind="ExternalInput").ap()
    w2T = nc.dram_tensor("w2T", [DIM, 2 * NP], F32, kind="ExternalInput").ap()
    b2 = nc.dram_tensor("b2", [2 * NP, 1], F32, kind="ExternalInput").ap()
    wpT = nc.dram_tensor("wpT", [INNER, DIM], F32, kind="ExternalInput").ap()
    bp = nc.dram_tensor("bp", [1, DIM], F32, kind="ExternalInput").ap()
    bk = nc.dram_tensor("bk", [1, INNER], F32, kind="ExternalInput").ap()
    bv = nc.dram_tensor("bv", [1, INNER], F32, kind="ExternalInput").ap()
    cst01 = nc.dram_tensor("cst01", [2, QPC], F32, kind="ExternalInput").ap()
    rep_in = nc.dram_tensor("rep_in", [16, 128], F32, kind="ExternalInput").ap()
    out_l = nc.dram_tensor("out_l", [QPC, DIM], F32, kind="ExternalOutput").ap()
    dbg = None
    if DEBUG:
        dbg = {
            "dbg_pxt": nc.dram_tensor("dbg_pxt", [128, 3], F32, kind="ExternalOutput").ap(),
            "dbg_sxy": nc.dram_tensor("dbg_sxy", [128, 16], F32, kind="ExternalOutput").ap(),
            "dbg_x0y0": nc.dram_tensor("dbg_x0y0", [128, 16], F32, kind="ExternalOutput").ap(),
            "dbg_wB": nc.dram_tensor("dbg_wB", [128, 16], F32, kind="ExternalOutput").ap(),
            "dbg_mi": nc.dram_tensor("dbg_mi", [128, 16], F32, kind="ExternalOutput").ap(),
            "dbg_kvraw": nc.dram_tensor("dbg_kvraw", [128, 512], F32, kind="ExternalOutput").ap(),
            "dbg_kvb": nc.dram_tensor("dbg_kvb", [128, 256], F32, kind="ExternalOutput").ap(),
            "dbg_sim": nc.dram_tensor("dbg_sim", [128, 32], F32, kind="ExternalOutput").ap(),
            "dbg_q": nc.dram_tensor("dbg_q", [128, 128], F32, kind="ExternalOutput").ap(),
        }

    with tile.TileContext(nc) as tc:
        _emit(tc, nc, lerp_op, img, wkvT, bev_l, wxy_l, ET, KT, wqT, bq,
              w1T, b1, w2T, b2, wpT, bp, bk, bv, cst01, rep_in, out_l, dbg)
    return nc


def _lerp(nc, lerp_op, out, in0, in1, s0, s1):
    """out = in0*s0 + in1*s1 with s0/s1 [P,1] columns."""
    if lerp_op is not None:
        nc.vector._custom_dve(lerp_op, out=out, in0=in0, in1=in1, s0=s0, s1=s1)
    else:
        # stock fallback: out = in0 + s1*(in1 - in0)   (valid since s0+s1=1)
        nc.vector.tensor_tensor(out=out, in0=in1, in1=in0,
                                op=mybir.AluOpType.subtract)
        nc.vector.tensor_scalar(out=out, in0=out, scalar1=s1, scalar2=None,
                                op0=mybir.AluOpType.mult)
        nc.vector.tensor_tensor(out=out, in0=out, in1=in0,
                                op=mybir.AluOpType.add)


def _emit(tc, nc, lerp_op, img, wkvT, bev_l, wxy_l, ET, KT, wqT, bq,
          w1T, b1, w2T, b2, wpT, bp, bk, bv, cst01, rep_in, out_l, dbg=None):
    import contextlib
    ctx = contextlib.ExitStack()
    with ctx:
        singles = ctx.enter_context(tc.tile_pool(name="singles", bufs=1))
        temps = ctx.enter_context(tc.tile_pool(name="temps", bufs=3))
        gath = ctx.enter_context(tc.tile_pool(name="gath", bufs=4))
        coords = ctx.enter_context(tc.tile_pool(name="coords", bufs=1))
        blend = ctx.enter_context(tc.tile_pool(name="blend", bufs=3))
        stats = ctx.enter_context(tc.tile_pool(name="stats", bufs=6))
        psum = ctx.enter_context(tc.tile_pool(name="psum", bufs=3, space="PSUM"))
        psum2 = ctx.enter_context(tc.tile_pool(name="psum2", bufs=2, space="PSUM"))
        dram = ctx.enter_context(tc.tile_pool(name="dram", bufs=1, space="DRAM"))

        AX = mybir.AxisListType
        ALU = mybir.AluOpType
        ACTF = mybir.ActivationFunctionType

        # ------------- resident tiles -------------
        ident = singles.tile([128, 128], F32)
        make_identity(nc, ident[:])
        wkvT_sb = singles.tile([DIM, KVROW], F32)
        nc.sync.dma_start(out=wkvT_sb[:], in_=wkvT)
        bev_sb = coords.tile([DIM, QPC], F32)
        nc.sync.dma_start(out=bev_sb[:], in_=bev_l)
        wqT_sb = singles.tile([DIM, INNER], F32)
        nc.sync.dma_start(out=wqT_sb[:], in_=wqT)
        w1T_sb = singles.tile([DIM, DIM], F32)
        nc.sync.dma_start(out=w1T_sb[:], in_=w1T)
        w2T_sb = singles.tile([DIM, 2 * NP], F32)
        nc.sync.dma_start(out=w2T_sb[:], in_=w2T)
        wpT_sb = singles.tile([INNER, DIM], F32)
        nc.sync.dma_start(out=wpT_sb[:], in_=wpT)
        b1_sb = singles.tile([DIM, 1], F32)
        nc.sync.dma_start(out=b1_sb[:], in_=b1)
        b2_sb = singles.tile([2 * NP, 1], F32)
        nc.sync.dma_start(out=b2_sb[:], in_=b2)
        ones_row = singles.tile([1, 128], F32)
        nc.vector.memset(ones_row[:], 1.0)

        def _rep128(name, src_ap, n):
            row = singles.tile([1, n], F32, tag=name + "_row")
            nc.sync.dma_start(out=row[:], in_=src_ap)
            ps = psum.tile([128, n], F32, tag="mm")
            nc.tensor.matmul(out=ps[:], lhsT=ones_row[:], rhs=row[:],
                             start=True, stop=True)
            t = singles.tile([128, n], F32, tag=name)
            nc.scalar.activation(out=t[:], in_=ps[:], func=ACTF.Copy)
            return t
        bq_sb = _rep128("bq128", bq, INNER)
        bp_sb = _rep128("bp128", bp, DIM)
        bk_sb = _rep128("bk128", bk, INNER)
        bv_sb = _rep128("bv128", bv, INNER)
        ET_sb = singles.tile([4, NCAM * 4], F32)
        nc.sync.dma_start(out=ET_sb[:], in_=ET)
        KT_sb = singles.tile([3, NCAM * 3], F32)
        nc.sync.dma_start(out=KT_sb[:], in_=KT)

        # DRAM scratch: paired-row kv table; row y*IW+x holds KV(y,x) ++ KV(y+1,x)
        kv_dram = dram.tile([NCAM, PADROWS, 2 * KVROW], BF16)
        # resident row indices (f32, exact ints): cols (cam, qt, r)
        idx2_all = singles.tile([128, NCAM * NQT * NP], F32)
        REP_sb = singles.tile([16, 128], F32)
        nc.sync.dma_start(out=REP_sb[:], in_=rep_in)

        # ------------- S1: kv tables -------------
        zt = singles.tile([128, KVROW], BF16)
        nc.vector.memset(zt[:], 0)
        NPT = IHW // 128  # 22 position tiles
        for cam in range(NCAM):
            # zero pads: first half rows >= IHW, second half rows >= IHW-IW
            nc.sync.dma_start(out=kv_dram[cam, IHW:PADROWS, 0:KVROW], in_=zt[:])
            nc.sync.dma_start(out=kv_dram[cam, IHW - IW:IHW - IW + 128, KVROW:2 * KVROW],
                              in_=zt[:])
            nc.sync.dma_start(out=kv_dram[cam, IHW - IW + 128:PADROWS, KVROW:2 * KVROW],
                              in_=zt[:PADROWS - (IHW - IW + 128), :])
            for pt in range(NPT):
                img_t = temps.tile([128, 128], F32, tag="imgt")
                nc.sync.dma_start(out=img_t[:], in_=img[cam, :, ts(pt, 128)])
                kv_ps = psum.tile([128, KVROW], F32, tag="mm")
                nc.tensor.matmul(out=kv_ps[:], lhsT=img_t[:], rhs=wkvT_sb[:],
                                 start=True, stop=True)
                kv_bf = temps.tile([128, KVROW], BF16, tag="kvbf")
                nc.scalar.activation(out=kv_bf[:], in_=kv_ps[:], func=ACTF.Copy)
                nc.sync.dma_start(out=kv_dram[cam, ts(pt, 128), 0:KVROW], in_=kv_bf[:])
                if pt == 0:
                    nc.sync.dma_start(out=kv_dram[cam, 0:128 - IW, KVROW:2 * KVROW],
                                      in_=kv_bf[IW:128, :])
                else:
                    nc.sync.dma_start(
                        out=kv_dram[cam, pt * 128 - IW:pt * 128 - IW + 128,
                                    KVROW:2 * KVROW],
                        in_=kv_bf[:])

        # ------------- S2/S3: queries, offsets, grid -------------
        # xyz1 = [wx, wy, 0, 1]
        xyz1_sb = coords.tile([4, QPC], F32)
        nc.sync.dma_start(out=xyz1_sb[:2, :], in_=wxy_l)
        nc.sync.dma_start(out=xyz1_sb[2:4, :], in_=cst01)

        # q projection: q_sb[q, ch] per q-tile; also bf16 copy and cq = q . bk
        q_sb = coords.tile([128, QPC], F32)        # [q-part, (qt,ch)]
        qbf_sb = singles.tile([128, QPC], BF16)
        cq_sb = singles.tile([128, NQT * HEADS], F32)
        for qt in range(NQT):
            q_ps = psum.tile([128, INNER], F32, tag="mm")
            nc.tensor.matmul(out=q_ps[:], lhsT=bev_sb[:, ts(qt, 128)],
                             rhs=wqT_sb[:], start=True, stop=True)
            nc.vector.tensor_tensor(out=q_sb[:, ts(qt, INNER)], in0=q_ps[:],
                                    in1=bq_sb[:],
                                    op=ALU.add)
            nc.scalar.activation(out=qbf_sb[:, ts(qt, INNER)],
                                 in_=q_sb[:, ts(qt, INNER)], func=ACTF.Copy)
            qbk = stats.tile([128, INNER], F32, tag="qbk")
            nc.vector.tensor_tensor(out=qbk[:], in0=q_sb[:, ts(qt, INNER)],
                                    in1=bk_sb[:],
                                    op=ALU.mult)
            nc.vector.tensor_reduce(
                out=cq_sb[:, ts(qt, HEADS)],
                in_=bass.AP(tensor=qbk.tensor, offset=qbk[:].offset,
                            ap=[qbk[:].ap[0], [DH, HEADS], [1, DH]]),
                axis=AX.X, op=ALU.add)

        # offsets: o1 = relu(w1 @ bev + b1); off = w2 @ o1 + b2  [16, QPC]
        o1_sb = coords.tile([DIM, QPC], F32)
        for hf in range(2):
            o1_ps = psum2.tile([DIM, QPC // 2], F32, tag="wide")
            nc.tensor.matmul(out=o1_ps[:], lhsT=w1T_sb[:],
                             rhs=bev_sb[:, ts(hf, QPC // 2)], start=True, stop=True)
            nc.scalar.activation(out=o1_sb[:, ts(hf, QPC // 2)], in_=o1_ps[:],
                                 func=ACTF.Relu, bias=b1_sb[:], scale=1.0)
        off_sb = coords.tile([2 * NP, QPC], F32)   # rows: c*8+p (x offs 0-7, y offs 8-15)
        for hf in range(2):
            off_ps = psum2.tile([2 * NP, QPC // 2], F32, tag="wide")
            nc.tensor.matmul(out=off_ps[:], lhsT=w2T_sb[:],
                             rhs=o1_sb[:, ts(hf, QPC // 2)], start=True, stop=True)
            nc.scalar.activation(out=off_sb[:, ts(hf, QPC // 2)], in_=off_ps[:],
                                 func=ACTF.Identity, bias=b2_sb[:], scale=1.0)

        # grid per camera -> packed coords -> transposed weights + indices
        # resident transposed weights: wA (wx0, wy0) and wB (wx1=fx, wy1=fy)
        # col layout: (n, qt) block of 16: [0:8]=x-part p, [8:16]=y-part p
        wA_sb = singles.tile([128, NCAM * NQT * 16], F32)
        wB_sb = singles.tile([128, NCAM * NQT * 16], F32)

        # transpose offsets once per q-tile: off_t_all [128, (qt, 16)]
        off_t_all = singles.tile([128, NQT * 16], F32)
        for qt in range(NQT):
            ot_ps = psum.tile([128, 2 * NP], F32, tag="mm")
            nc.tensor.transpose(out=ot_ps[:], in_=off_sb[:, ts(qt, 128)],
                                identity=ident[:2 * NP, :2 * NP])
            nc.scalar.activation(out=off_t_all[:, ts(qt, 2 * NP)], in_=ot_ps[:],
                                 func=ACTF.Copy)

        for cam in range(NCAM):
            xyzw_sb = coords.tile([4, QPC], F32, tag="xyzw")
            for hf in range(2):
                xyzw_ps = psum2.tile([4, QPC // 2], F32, tag="wide")
                nc.tensor.matmul(out=xyzw_ps[:],
                                 lhsT=ET_sb[:, ts(cam, 4)], rhs=xyz1_sb[:, ts(hf, QPC // 2)],
                                 start=True, stop=True)
                nc.scalar.activation(out=xyzw_sb[:, ts(hf, QPC // 2)], in_=xyzw_ps[:],
                                     func=ACTF.Copy)
            pix_sb = coords.tile([3, QPC], F32, tag="pix")
            for hf in range(2):
                pix_ps = psum2.tile([3, QPC // 2], F32, tag="wide")
                nc.tensor.matmul(out=pix_ps[:],
                                 lhsT=KT_sb[:, ts(cam, 3)], rhs=xyzw_sb[:3, ts(hf, QPC // 2)],
                                 start=True, stop=True)
                nc.scalar.activation(out=pix_sb[:, ts(hf, QPC // 2)], in_=pix_ps[:],
                                     func=ACTF.Copy)
            camrow = cam * PADROWS
            for qt in range(NQT):
                # pix^T for this q-tile -> [128, 3]
                pt_ps = psum.tile([128, 3], F32, tag="mm")
                nc.tensor.transpose(out=pt_ps[:], in_=pix_sb[:, ts(qt, 128)],
                                    identity=ident[:3, :3])
                pxt = stats.tile([128, 3], F32, tag="pxt")
                nc.scalar.activation(out=pxt[:], in_=pt_ps[:], func=ACTF.Copy)
                rden = stats.tile([128, 1], F32, tag="rden")
                nc.vector.tensor_scalar(out=rden[:], in0=pxt[:, 2:3], scalar1=1e-6,
                                        scalar2=None, op0=ALU.max)
                nc.vector.reciprocal(out=rden[:], in_=rden[:])
                gxy = stats.tile([128, 2], F32, tag="gxy")
                nc.vector.tensor_scalar(out=gxy[:], in0=pxt[:, 0:2], scalar1=rden[:],
                                        scalar2=None, op0=ALU.mult)
                nc.vector.tensor_scalar(out=gxy[:, 0:1], in0=gxy[:, 0:1],
                                        scalar1=2.0 / (IW - 1), scalar2=1.0,
                                        op0=ALU.mult, op1=ALU.subtract)
                nc.vector.tensor_scalar(out=gxy[:, 1:2], in0=gxy[:, 1:2],
                                        scalar1=2.0 / (IH - 1), scalar2=1.0,
                                        op0=ALU.mult, op1=ALU.subtract)
                blkw = (cam * NQT + qt) * 16
                x0y0 = stats.tile([128, 2 * NP], F32, tag="x0y0")
                sxy = stats.tile([128, 2 * NP], F32, tag="sxy")
                for c in range(2):  # 0 -> x, 1 -> y
                    cs = slice(c * NP, c * NP + NP)
                    half = 0.5 * ((IW - 1) if c == 0 else (IH - 1))
                    # samp = clip(off + g, -1, 1); i = (samp + 1) * half
                    nc.vector.tensor_scalar(out=sxy[:, cs],
                                            in0=off_t_all[:, qt * 16 + c * NP:
                                                          qt * 16 + c * NP + NP],
                                            scalar1=gxy[:, c:c + 1], scalar2=None,
                                            op0=ALU.add)
                    nc.vector.tensor_scalar(out=sxy[:, cs], in0=sxy[:, cs],
                                            scalar1=1.0, scalar2=-1.0,
                                            op0=ALU.min, op1=ALU.max)
                    nc.vector.tensor_scalar(out=sxy[:, cs], in0=sxy[:, cs],
                                            scalar1=1.0, scalar2=half,
                                            op0=ALU.add, op1=ALU.mult)
                # floor via +2^23 round-to-nearest, then fixup so frac >= 0
                BIGF = 8388608.0
                rnd = stats.tile([128, 2 * NP], F32, tag="rnd")
                nc.vector.tensor_scalar(out=rnd[:], in0=sxy[:], scalar1=BIGF,
                                        scalar2=None, op0=ALU.add)
                nc.vector.tensor_scalar(out=rnd[:], in0=rnd[:], scalar1=BIGF,
                                        scalar2=None, op0=ALU.subtract)
                dfr = stats.tile([128, 2 * NP], F32, tag="dfr")
                nc.vector.tensor_tensor(out=dfr[:], in0=sxy[:], in1=rnd[:],
                                        op=ALU.subtract)
                msk = stats.tile([128, 2 * NP], F32, tag="msk")
                nc.vector.tensor_scalar(out=msk[:], in0=dfr[:], scalar1=0.0,
                                        scalar2=None, op0=ALU.is_lt)
                nc.vector.tensor_tensor(out=x0y0[:], in0=rnd[:], in1=msk[:],
                                        op=ALU.subtract)
                # frac -> wB, 1 - frac -> wA
                nc.vector.tensor_tensor(out=wB_sb[:, blkw:blkw + 16], in0=dfr[:],
                                        in1=msk[:], op=ALU.add)
                nc.vector.tensor_scalar(out=wA_sb[:, blkw:blkw + 16],
                                        in0=wB_sb[:, blkw:blkw + 16],
                                        scalar1=-1.0, scalar2=1.0,
                                        op0=ALU.mult, op1=ALU.add)
                # idx = y0*IW + x0 ; Mi32 even = idx*KVROW + camoff, odd = +IW*KVROW
                idxf = stats.tile([128, NP], F32, tag="idxf")
                nc.vector.tensor_scalar(out=idxf[:], in0=x0y0[:, NP:2 * NP],
                                        scalar1=float(IW), scalar2=None, op0=ALU.mult)
                nc.vector.tensor_tensor(out=idxf[:], in0=idxf[:], in1=x0y0[:, 0:NP],
                                        op=ALU.add)
                base = (cam * NQT + qt) * NP
                nc.vector.tensor_scalar(out=idx2_all[:, base:base + NP], in0=idxf[:],
                                        scalar1=float(camrow), scalar2=None,
                                        op0=ALU.add)
                if dbg is not None and cam == 0 and qt == 0:
                    nc.sync.dma_start(out=dbg["dbg_pxt"], in_=pxt[:])
                    nc.sync.dma_start(out=dbg["dbg_sxy"], in_=sxy[:])
                    nc.sync.dma_start(out=dbg["dbg_x0y0"], in_=x0y0[:])
                    nc.sync.dma_start(out=dbg["dbg_wB"], in_=wB_sb[:, blkw:blkw + 16])
                    nc.sync.dma_start(out=dbg["dbg_mi"], in_=idx2_all[:, 0:16])

        # ------------- S5: main attention loop -------------
        kv_flat = kv_dram[:]
        NROWS = NCAM * PADROWS - 1
        for qt in range(NQT):
            # build 16-wrapped idx table T_qt[16k+pl, cam*128 + r*8 + qh]
            #   = idx2_all[qh*16+pl, (cam, qt, r)]
            T16f = coords.tile([16, NCAM * 64], F32, tag="T16f")
            # one transpose of the 48 idx cols (cam, r) for this qt
            ia = idx2_all[:]
            idx_slice = bass.AP(tensor=idx2_all.tensor, offset=ia.offset + qt * NP,
                                ap=[ia.ap[0], [NQT * NP, NCAM], [1, NP]])
            idx_c = stats.tile([128, NCAM * NP], F32, tag="idxc")
            nc.vector.tensor_copy(out=idx_c[:], in_=idx_slice)
            ta_ps = psum.tile([48, 128], F32, tag="mm")
            nc.tensor.transpose(out=ta_ps[:], in_=idx_c[:], identity=ident[:])
            tas = coords.tile([48, 128], F32, tag="tas")
            nc.scalar.activation(out=tas[:], in_=ta_ps[:], func=ACTF.Copy)
            for qh in range(8):
                tb_ps = psum.tile([16, 48], F32, tag="mm")
                nc.tensor.transpose(out=tb_ps[:], in_=tas[:, 16 * qh:16 * qh + 16],
                                    identity=ident[:48, :48])
                t16ap = T16f[:]
                dst = bass.AP(tensor=T16f.tensor, offset=t16ap.offset + qh,
                              ap=[t16ap.ap[0], [64, NCAM], [8, NP]])
                nc.scalar.activation(out=dst, in_=tb_ps[:], func=ACTF.Copy)
            T_qt = gath.tile([128, NCAM * 64], I16, tag="Tqt")
            rep_ps = psum2.tile([128, NCAM * 64], F32, tag="wide")
            nc.tensor.matmul(out=rep_ps[:], lhsT=REP_sb[:],
                             rhs=T16f[:], start=True, stop=True)
            nc.vector.tensor_copy(out=T_qt[:], in_=rep_ps[:])
            wacc = stats.tile([128, INNER], F32, tag="wacc")
            nc.vector.memset(wacc[:], 0.0)
            for cam in range(NCAM):
                kvraw = gath.tile([128, NP, 4 * KVROW], BF16, tag="kvraw")
                kv_view = bass.AP(tensor=kv_dram.tensor, offset=0,
                                  ap=[[2 * KVROW, NROWS], [1, 4 * KVROW]])
                nc.gpsimd.dma_gather(
                    out_ap=kvraw[:], in_ap=kv_view,
                    idxs_ap=T_qt[:, ts(cam, 64)],
                    num_idxs=1024, num_idxs_reg=1024,
                    elem_size=4 * KVROW, elem_step=2 * KVROW,
                    single_packet=False)
                # x-blend: 8 rows of 512 [(y0,y1) x (K|V)]
                blkw = (cam * NQT + qt) * 16
                kvx = blend.tile([128, NP, 2 * KVROW], BF16, tag="kvx")
                for p in range(NP):
                    _lerp(nc, lerp_op, kvx[:, p, :],
                          kvraw[:, p, 0:2 * KVROW], kvraw[:, p, 2 * KVROW:4 * KVROW],
                          wA_sb[:, blkw + p:blkw + p + 1],
                          wB_sb[:, blkw + p:blkw + p + 1])
                # y-blend: 8 points of 256
                kvb = blend.tile([128, NP, KVROW], BF16, tag="kvb")
                for p in range(NP):
                    _lerp(nc, lerp_op, kvb[:, p, :],
                          kvx[:, p, 0:KVROW], kvx[:, p, KVROW:2 * KVROW],
                          wA_sb[:, blkw + 8 + p:blkw + 9 + p],
                          wB_sb[:, blkw + 8 + p:blkw + 9 + p])
                # K dot q -> sim [128, p, h]
                up = blend.tile([128, NP, INNER], BF16, tag="up")
                qv = qbf_sb[:, ts(qt, INNER)]
                nc.vector.tensor_tensor(
                    out=up[:], in0=kvb[:, :, 0:INNER],
                    in1=bass.AP(tensor=qbf_sb.tensor, offset=qv.offset,
                                ap=[qv.ap[0], [0, NP], [1, INNER]]),
                    op=ALU.mult)
                if dbg is not None and cam == 0 and qt == 0:
                    kvraw_f = temps.tile([128, 512], F32, tag="kvrawf")
                    nc.scalar.activation(out=kvraw_f[:], in_=kvraw[:, 0, :], func=ACTF.Copy)
                    nc.sync.dma_start(out=dbg["dbg_kvraw"], in_=kvraw_f[:])
                    kvb_f = temps.tile([128, 256], F32, tag="kvbf2")
                    nc.scalar.activation(out=kvb_f[:], in_=kvb[:, 0, :], func=ACTF.Copy)
                    nc.sync.dma_start(out=dbg["dbg_kvb"], in_=kvb_f[:])
                sim = stats.tile([128, NP, HEADS], F32, tag="sim")
                upap = up[:]
                nc.vector.tensor_reduce(
                    out=sim[:],
                    in_=bass.AP(tensor=up.tensor, offset=upap.offset,
                                ap=[upap.ap[0], [INNER, NP], [DH, HEADS], [1, DH]]),
                    axis=AX.X, op=ALU.add)
                if dbg is not None and cam == 0 and qt == 0:
                    nc.sync.dma_start(out=dbg["dbg_sim"], in_=sim[:])
                    nc.sync.dma_start(out=dbg["dbg_q"], in_=q_sb[:, 0:128])
                cqv = cq_sb[:, ts(qt, HEADS)]
                nc.vector.tensor_tensor(
                    out=sim[:], in0=sim[:],
                    in1=bass.AP(tensor=cq_sb.tensor, offset=cqv.offset,
                                ap=[cqv.ap[0], [0, NP], [1, HEADS]]),
                    op=ALU.add)
                # softmax over p (and fold the 1/NCAM mean)
                esim = stats.tile([128, NP, HEADS], F32, tag="esim")
                nc.scalar.activation(out=esim[:], in_=sim[:], func=ACTF.Exp)
                ssum = stats.tile([128, HEADS], F32, tag="ssum")
                esap = esim[:]
                nc.vector.tensor_reduce(
                    out=ssum[:],
                    in_=bass.AP(tensor=esim.tensor, offset=esap.offset,
                                ap=[esap.ap[0], [1, HEADS], [HEADS, NP]]),
                    axis=AX.X, op=ALU.add)
                nc.vector.tensor_scalar(out=ssum[:], in0=ssum[:],
                                        scalar1=float(NCAM), scalar2=None,
                                        op0=ALU.mult)
                srec = stats.tile([128, HEADS], F32, tag="srec")
                nc.vector.reciprocal(out=srec[:], in_=ssum[:])
                att = stats.tile([128, NP, HEADS], BF16, tag="att")
                srap = srec[:]
                nc.vector.tensor_tensor(
                    out=att[:], in0=esim[:],
                    in1=bass.AP(tensor=srec.tensor, offset=srap.offset,
                                ap=[srap.ap[0], [0, NP], [1, HEADS]]),
                    op=ALU.mult)
                # weighted V sum over p
                vw = blend.tile([128, NP, INNER], BF16, tag="vw")
                atap = att[:]
                nc.vector.tensor_tensor(
                    out=vw[:], in0=kvb[:, :, INNER:KVROW],
                    in1=bass.AP(tensor=att.tensor, offset=atap.offset,
                                ap=[atap.ap[0], [HEADS, NP], [1, HEADS], [0, DH]]),
                    op=ALU.mult)
                wsum = stats.tile([128, INNER], F32, tag="wsum")
                vwap = vw[:]
                nc.vector.tensor_reduce(
                    out=wsum[:],
                    in_=bass.AP(tensor=vw.tensor, offset=vwap.offset,
                                ap=[vwap.ap[0], [1, INNER], [INNER, NP]]),
                    axis=AX.X, op=ALU.add)
                nc.vector.tensor_tensor(out=wacc[:], in0=wacc[:], in1=wsum[:],
                                        op=ALU.add)
            # + bv, then output projection
            nc.vector.tensor_tensor(out=wacc[:], in0=wacc[:],
                                    in1=bv_sb[:],
                                    op=ALU.add)
            wt_ps = psum.tile([128, 128], F32, tag="mm")
            nc.tensor.transpose(out=wt_ps[:], in_=wacc[:], identity=ident[:])
            waccT = temps.tile([128, 128], F32, tag="waccT")
            nc.scalar.activation(out=waccT[:], in_=wt_ps[:], func=ACTF.Copy)
            out_ps = psum.tile([128, DIM], F32, tag="mm")
            nc.tensor.matmul(out=out_ps[:], lhsT=waccT[:], rhs=wpT_sb[:],
                             start=True, stop=True)
            outf = temps.tile([128, DIM], F32, tag="outf")
            nc.vector.tensor_tensor(out=outf[:], in0=out_ps[:],
                                    in1=bp_sb[:],
                                    op=ALU.add)
            nc.sync.dma_start(out=out_l[ts(qt, 128), :], in_=outf[:])


# ---------------------------------------------------------------- host side
_CACHED = {}


def _build():
    if "nc" not in _CACHED:
        nc = bacc.Bacc("TRN2", target_bir_lowering=False, debug=False,
                       num_devices=NCORES)
        build_kernel(nc)
        nc.compile()
        _CACHED["nc"] = nc
    return _CACHED["nc"]


def make_in_maps(inputs):
    """Slice/transpose FULL inputs into 8 per-core input dicts (layout only)."""
    f = lambda x: np.ascontiguousarray(np.asarray(x, dtype=np.float32))
    bev = f(inputs["bev"]).reshape(B, DIM, HW)
    img_feats = f(inputs["img_feats"]).reshape(B, NCAM, DIM, IHW)
    Kc = f(inputs["K"])
    Ec = f(inputs["E"])
    world_xy = f(inputs["world_xy"]).reshape(2, HW)
    wq = f(inputs["wq"]); bq = f(inputs["bq"])
    wkv = f(inputs["wkv"]); bkv = f(inputs["bkv"])
    w_off1 = f(inputs["w_off1"]); b_off1 = f(inputs["b_off1"])
    w_off2 = f(inputs["w_off2"]); b_off2 = f(inputs["b_off2"])
    w_proj = f(inputs["w_proj"]); b_proj = f(inputs["b_proj"])

    # row-permute w_off2/b_off2 from (p, c) to (c, p) ordering
    perm = [p * 2 + c for c in range(2) for p in range(NP)]
    w2p = w_off2[perm, :]
    b2p = b_off2[perm]

    in_maps = []
    for core in range(NCORES):
        bc = core // (NCORES // B)
        q0 = (core % (NCORES // B)) * QPC
        m = {
            "img": np.ascontiguousarray(img_feats[bc]),
            "wkvT": np.ascontiguousarray(wkv.T),
            "bev_l": np.ascontiguousarray(bev[bc, :, q0:q0 + QPC]),
            "wxy_l": np.ascontiguousarray(world_xy[:, q0:q0 + QPC]),
            "ET": np.ascontiguousarray(Ec[bc].transpose(2, 0, 1).reshape(4, NCAM * 4)),
            "KT": np.ascontiguousarray(Kc[bc].transpose(2, 0, 1).reshape(3, NCAM * 3)),
            "wqT": np.ascontiguousarray(wq.T),
            "bq": bq.reshape(1, INNER),
            "w1T": np.ascontiguousarray(w_off1.T),
            "b1": b_off1.reshape(DIM, 1),
            "w2T": np.ascontiguousarray(w2p.T),
            "b2": b2p.reshape(2 * NP, 1),
            "wpT": np.ascontiguousarray(w_proj.T),
            "bp": b_proj.reshape(1, DIM),
            "bk": bkv[:INNER].reshape(1, INNER),
            "bv": bkv[INNER:].reshape(1, INNER),
            "cst01": np.concatenate([np.zeros((1, QPC), np.float32),
                                     np.ones((1, QPC), np.float32)], 0),
            "rep_in": (np.arange(128)[None, :] % 16 ==
                       np.arange(16)[:, None]).astype(np.float32),
            "out_l": None,
        }
        m.pop("out_l")
        in_maps.append(m)
    return in_maps


def assemble(results):
    """results: list of 8 dicts with out_l [QPC, DIM] -> [B, DIM, H, W]."""
    full = np.zeros((B, HW, DIM), dtype=np.float32)
    for core, r in enumerate(results):
        bc = core // (NCORES // B)
        q0 = (core % (NCORES // B)) * QPC
        full[bc, q0:q0 + QPC, :] = r["out_l"]
    return np.ascontiguousarray(full.transpose(0, 2, 1).reshape(B, DIM, H, W))


def kernel(**inputs):
    from concourse.bass_utils import run_bass_kernel_spmd
    nc = _build()
    in_maps = make_in_maps(inputs)
    res = run_bass_kernel_spmd(nc, in_maps, core_ids=list(range(NCORES)))
    return assemble(res.results)


if __name__ == "__main__":
    import reference
    inputs = {k: np.asarray(v) for k, v in reference.setup_inputs().items()}
    out = kernel(**inputs)
    exp = np.asarray(reference.reference(**{k: np.asarray(v) for k, v in inputs.items()}))
    err = np.abs(out - exp).max() / (np.abs(exp).max() + 1e-9)
    print("Relative error:", err)



# revision 2
# speedup vs baseline: 1.2159x; 1.2159x over previous
"""Trainium2 Bass kernel for CrossViewDeformableBlock (sparse deformable attention).

Contract: kernel(**inputs) -> np.ndarray takes FULL inputs (as from
setup_inputs()) and returns the FULL output [b, 128, 64, 64].

Sharding: 8 cores, q-parallel. Core c handles batch b_c = c//4 and query
range [(c%4)*1024, +1024) of the 64*64=4096 BEV queries. Each core builds
the bf16 K|V image tables for its 6 cameras on-device, computes projection
/ offsets / bilinear sample coordinates on-device (batched across all
(cam, q-tile) pairs), gathers 2-row pairs with dma_gather, blends corners
with a custom DVE lerp that runs in the 2X_1PORT perf mode, does the
point-softmax attention and output projection, and writes its [1024, 128]
output shard. The host only slices inputs, transposes weights (layout),
and concatenates shards.
"""

import math
import os
import numpy as np

import concourse.bass as bass
import concourse.mybir as mybir
import concourse.tile as tile
from concourse import bacc
from concourse.bass import ts
from concourse.masks import make_identity

# ---------------------------------------------------------------- constants
B, NCAM, H, W = 2, 6, 64, 64
HW = H * W                      # 4096 queries per batch
IH, IW = 32, 88                 # image feature h, w
IHW = IH * IW                   # 2816 positions
HEADS, DH, INNER = 4, 32, 128
NP = 8                          # sample points per query
DIM = 128
NCORES = 8
QPC = HW // (NCORES // B)       # 1024 queries per core
NQT = QPC // 128                # 8 q-tiles of 128
PADROWS = 2944                  # 23 * 128 rows in kv table (2816 + 128 pad)
KVROW = 2 * INNER               # 256 channels (K|V) per position
NCQ = NCAM * NQT                # 48 (cam, qtile) pairs
F32 = mybir.dt.float32
BF16 = mybir.dt.bfloat16
I16 = mybir.dt.int16
I32 = mybir.dt.int32

_USE_CUSTOM_LERP = True


def _register_lerp_op():
    """Register LERP2: out = in0*s0 + in1*s1 (per-partition scalars s0,s1).

    Registers both the 1x program (from lower()) and a hand-written
    2X_1PORT program so bf16 step-1 operands run at 2 elems/cycle when the
    emitted instruction sets perf_max.
    """
    from concourse.dve_spec import Spec, Src0, Src1, C0, C1, lower
    from concourse.dve_uop import (DveOpSpec, UopConfig, UopDpConfig, InpSel,
                                   AluInp, DelayInp, OutSel, OutPath, Trigger)
    from concourse.dve_spec import AluOp as DAlu
    from concourse.dve_ops import DveOp, OPS, _SUB_OPCODE_FOR_NAME, \
        _CUSTOM_DVE_ROW_BASE, _COMPILE_CACHE

    name = "LERP2"
    if name in _SUB_OPCODE_FOR_NAME:
        for op in OPS:
            if op.name == name:
                return op
    spec = Spec(
        body=Src0 * C0 + Src1 * C1,
        reference=lambda in0, in1, s0, s1, imm2: (
            in0.astype(np.float32) * s0 + in1.astype(np.float32) * s1
        ),
    )
    opcode = _CUSTOM_DVE_ROW_BASE + len(OPS)
    assert opcode < 0x20

    uops_1x = lower(spec, ver="v3")

    PD = DelayInp.PREV_DELAY
    PA = DelayInp.PREV_ALU_OUT
    A = AluInp
    MUL, ADD, BYP = DAlu.MULTIPLY, DAlu.ADD, DAlu.BYPASS

    def dp(op_, s0_, s1_, delay_sel, delay_en):
        return UopDpConfig(
            op=op_, alu_src0=s0_, alu_src1=s1_,
            delay=list(delay_sel), alu_out_enable=1, swap_enable=0,
            alu_out_a_enable=0, alu_out_b_enable=0,
            delay_enable=list(delay_en), idx0_sel=0, idx1_sel=0)

    # lanes: 0=SRC_0 1=SRC_1 2=SRC_0_HI 3=SRC_1_HI 4=CONST_0 5=CONST_1
    # stage-0 delay load: d_k <- lane k+1 (sel=PREV_DELAY)
    # => d0=SRC_1 d1=SRC_0_HI d2=SRC_1_HI d3=C0 d4=C1
    stages = [
        # m0l = SRC_0 (lane0) * C0 (d3)
        dp(MUL, A.PREV_ALU_OUT, A.PREV_DELAY_3,
           [PD, PD, PD, PD, PD, PA, PA], [1, 1, 1, 1, 1, 0, 0]),
        # m1l = SRC_1 (d0) * C1 (d4); d0 <- m0l
        dp(MUL, A.PREV_DELAY_0, A.PREV_DELAY_4,
           [PA, PD, PD, PD, PD, PA, PA], [1, 1, 1, 1, 1, 0, 0]),
        # lo = m0l (d0) + m1l (prev alu)
        dp(ADD, A.PREV_DELAY_0, A.PREV_ALU_OUT,
           [PA, PD, PD, PD, PD, PA, PA], [0, 1, 1, 1, 1, 0, 0]),
        # m0h = SRC_0_HI (d1) * C0 (d3); d0 <- lo
        dp(MUL, A.PREV_DELAY_1, A.PREV_DELAY_3,
           [PA, PA, PD, PD, PD, PA, PA], [1, 0, 1, 1, 1, 0, 0]),
        # m1h = SRC_1_HI (d2) * C1 (d4); d0 keep lo; d1 <- m0h
        dp(MUL, A.PREV_DELAY_2, A.PREV_DELAY_4,
           [PD, PA, PA, PA, PA, PA, PA], [1, 1, 0, 0, 0, 0, 0]),
        # hi = m0h (d1) + m1h (prev alu); d0 keep lo
        dp(ADD, A.PREV_DELAY_1, A.PREV_ALU_OUT,
           [PD, PA, PA, PA, PA, PA, PA], [1, 0, 0, 0, 0, 0, 0]),
        # out chain <- lo (d0); d0 <- hi
        dp(BYP, A.PREV_DELAY_0, A.PREV_ALU_OUT,
           [PA, PA, PA, PA, PA, PA, PA], [1, 0, 0, 0, 0, 0, 0]),
        # carry lo on alu chain; keep hi in d0
        dp(BYP, A.PREV_ALU_OUT, A.PREV_ALU_OUT,
           [PD, PA, PA, PA, PA, PA, PA], [1, 0, 0, 0, 0, 0, 0]),
    ]
    uop2x = UopConfig(
        inp=[InpSel.SRC_0, InpSel.SRC_1, InpSel.SRC_0_HI, InpSel.SRC_1_HI,
             InpSel.CONST_0, InpSel.CONST_1, InpSel.ZERO, InpSel.ZERO],
        inp_enable=[1, 1, 1, 1, 1, 1, 0, 0],
        out={OutPath.WR0_LO: OutSel.ALU_OUT, OutPath.WR0_HI: OutSel.DELAY_0,
             OutPath.WR1_LO: OutSel.ALU_OUT, OutPath.WR1_HI: OutSel.ALU_OUT},
        out_enable={OutPath.WR0_LO: 1, OutPath.WR0_HI: 1,
                    OutPath.WR1_LO: 0, OutPath.WR1_HI: 0},
        out_last_subdim_enable=0,
        force_two_data_zero=0, force_two_data_one=0,
        require_inp0=1, require_inp1=1, repeat_count=0,
        trigger=(Trigger.SRC_TENSOR_DONE, Trigger.NONE, Trigger.NONE),
        next_uop=(0, 0, 0),
        inc_parameter_index=0, enable_rev_ops=0, match_mask=0, valid_match=0,
        replace_on_match=0, clear_match=0, write_predicate_select=0,
        write_predicate_enable=0, delay_shift8=0, index_increment=0,
        index_clear=0, accum_enabled=0, v4={},
        datapath_config=stages,
    )
    full = DveOpSpec(name=name, opcode=opcode, uops=uops_1x,
                     uops_2x=[uop2x], rd1_en=True, perf_max=1)
    full.validate("v3")
    op = DveOp(name, spec, subdim=False, uops_sha={"v3": full.sha("v3")},
               perf_en={"v3": True})
    OPS.append(op)
    _SUB_OPCODE_FOR_NAME[name] = opcode
    from concourse import dve_ops as _do
    _do.CUSTOM_DVE_SPECS[name] = spec
    _do._COMPILE_CACHE[(name, "v3")] = full
    return op


def _lerp(nc, lerp_op, out, in0, in1, s0, s1):
    """out = in0*s0 + in1*s1 with s0/s1 [P,1] columns (bf16 data, 2x mode)."""
    if lerp_op is not None:
        inst = nc.vector._custom_dve(lerp_op, out=out, in0=in0, in1=in1,
                                     s0=s0, s1=s1)
        inst.ins.perf_max = 1
    else:
        nc.vector.tensor_tensor(out=out, in0=in1, in1=in0,
                                op=mybir.AluOpType.subtract)
        nc.vector.tensor_scalar(out=out, in0=out, scalar1=s1, scalar2=None,
                                op0=mybir.AluOpType.mult)
        nc.vector.tensor_tensor(out=out, in0=out, in1=in0,
                                op=mybir.AluOpType.add)


def build_kernel(nc):
    """Emit the SPMD program. All per-core variation comes via input data."""
    lerp_op = _register_lerp_op() if _USE_CUSTOM_LERP else None

    # ---------------- dram I/O ----------------
    img = nc.dram_tensor("img", [NCAM, DIM, IHW], BF16, kind="ExternalInput").ap()
    wkvT = nc.dram_tensor("wkvT", [DIM, KVROW], BF16, kind="ExternalInput").ap()
    bev_l = nc.dram_tensor("bev_l", [DIM, QPC], BF16, kind="ExternalInput").ap()
    wxy_l = nc.dram_tensor("wxy_l", [2, QPC], F32, kind="ExternalInput").ap()
    MT = nc.dram_tensor("MT", [4, NCAM * 3], F32, kind="ExternalInput").ap()
    wqT = nc.dram_tensor("wqT", [DIM, INNER], BF16, kind="ExternalInput").ap()
    bq = nc.dram_tensor("bq", [1, INNER], F32, kind="ExternalInput").ap()
    w1T = nc.dram_tensor("w1T", [DIM, DIM], BF16, kind="ExternalInput").ap()
    b1 = nc.dram_tensor("b1", [DIM, 1], F32, kind="ExternalInput").ap()
    w2T = nc.dram_tensor("w2T", [DIM, 2 * NP], BF16, kind="ExternalInput").ap()
    b2 = nc.dram_tensor("b2", [2 * NP, 1], F32, kind="ExternalInput").ap()
    wpT = nc.dram_tensor("wpT", [INNER, DIM], F32, kind="ExternalInput").ap()
    bp = nc.dram_tensor("bp", [1, DIM], F32, kind="ExternalInput").ap()
    bk = nc.dram_tensor("bk", [1, INNER], F32, kind="ExternalInput").ap()
    bv = nc.dram_tensor("bv", [1, INNER], F32, kind="ExternalInput").ap()
    cst01 = nc.dram_tensor("cst01", [2, QPC], F32, kind="ExternalInput").ap()
    rep_in = nc.dram_tensor("rep_in", [16, 128], F32, kind="ExternalInput").ap()
    out_l = nc.dram_tensor("out_l", [QPC, DIM], F32, kind="ExternalOutput").ap()

    with tile.TileContext(nc) as tc:
        _emit(tc, nc, lerp_op, img, wkvT, bev_l, wxy_l, MT, wqT, bq,
              w1T, b1, w2T, b2, wpT, bp, bk, bv, cst01, rep_in, out_l)
    return nc


def _emit(tc, nc, lerp_op, img, wkvT, bev_l, wxy_l, MT, wqT, bq,
          w1T, b1, w2T, b2, wpT, bp, bk, bv, cst01, rep_in, out_l):
    import contextlib
    ctx = contextlib.ExitStack()
    with ctx:
        singles = ctx.enter_context(tc.tile_pool(name="singles", bufs=1))
        temps = ctx.enter_context(tc.tile_pool(name="temps", bufs=3))
        gath = ctx.enter_context(tc.tile_pool(name="gath", bufs=4))
        coords = ctx.enter_context(tc.tile_pool(name="coords", bufs=1))
        blend = ctx.enter_context(tc.tile_pool(name="blend", bufs=3))
        stats = ctx.enter_context(tc.tile_pool(name="stats", bufs=6))
        psum = ctx.enter_context(tc.tile_pool(name="psum", bufs=3, space="PSUM"))
        psum2 = ctx.enter_context(tc.tile_pool(name="psum2", bufs=2, space="PSUM"))
        dram = ctx.enter_context(tc.tile_pool(name="dram", bufs=1, space="DRAM"))

        AX = mybir.AxisListType
        ALU = mybir.AluOpType
        ACTF = mybir.ActivationFunctionType

        # ------------- resident tiles -------------
        ident = singles.tile([128, 128], F32)
        make_identity(nc, ident[:])
        wkvT_sb = singles.tile([DIM, KVROW], BF16)
        nc.sync.dma_start(out=wkvT_sb[:], in_=wkvT)
        bev_sb = coords.tile([DIM, QPC], BF16)
        nc.sync.dma_start(out=bev_sb[:], in_=bev_l)
        wqT_sb = singles.tile([DIM, INNER], BF16)
        nc.sync.dma_start(out=wqT_sb[:], in_=wqT)
        w1T_sb = singles.tile([DIM, DIM], BF16)
        nc.sync.dma_start(out=w1T_sb[:], in_=w1T)
        w2T_sb = singles.tile([DIM, 2 * NP], BF16)
        nc.sync.dma_start(out=w2T_sb[:], in_=w2T)
        wpT_sb = singles.tile([INNER, DIM], F32)
        nc.sync.dma_start(out=wpT_sb[:], in_=wpT)
        b1_sb = singles.tile([DIM, 1], F32)
        nc.sync.dma_start(out=b1_sb[:], in_=b1)
        b2_sb = singles.tile([2 * NP, 1], F32)
        nc.sync.dma_start(out=b2_sb[:], in_=b2)
        ones_row = singles.tile([1, 128], F32)
        nc.vector.memset(ones_row[:], 1.0)

        def _rep128(name, src_ap, n):
            row = singles.tile([1, n], F32, tag=name + "_row")
            nc.sync.dma_start(out=row[:], in_=src_ap)
            ps = psum.tile([128, n], F32, tag="mm")
            nc.tensor.matmul(out=ps[:], lhsT=ones_row[:], rhs=row[:],
                             start=True, stop=True)
            t = singles.tile([128, n], F32, tag=name)
            nc.scalar.activation(out=t[:], in_=ps[:], func=ACTF.Copy)
            return t
        bq_sb = _rep128("bq128", bq, INNER)
        bp_sb = _rep128("bp128", bp, DIM)
        bk_sb = _rep128("bk128", bk, INNER)
        bv_sb = _rep128("bv128", bv, INNER)
        MT_sb = singles.tile([4, NCAM * 3], F32)
        nc.sync.dma_start(out=MT_sb[:], in_=MT)

        # DRAM scratch: paired-row kv table; row y*IW+x holds KV(y,x) ++ KV(y+1,x)
        kv_dram = dram.tile([NCAM, PADROWS, 2 * KVROW], BF16)
        REP_sb = singles.tile([16, 128], F32)
        nc.sync.dma_start(out=REP_sb[:], in_=rep_in)

        # ------------- S1: kv tables -------------
        zt = singles.tile([128, KVROW], BF16)
        nc.vector.memset(zt[:], 0)
        NPT = IHW // 128  # 22 position tiles
        for cam in range(NCAM):
            nc.sync.dma_start(out=kv_dram[cam, IHW:PADROWS, 0:KVROW], in_=zt[:])
            nc.sync.dma_start(out=kv_dram[cam, IHW - IW:IHW - IW + 128, KVROW:2 * KVROW],
                              in_=zt[:])
            nc.sync.dma_start(out=kv_dram[cam, IHW - IW + 128:PADROWS, KVROW:2 * KVROW],
                              in_=zt[:PADROWS - (IHW - IW + 128), :])
            for pt in range(NPT):
                img_t = temps.tile([128, 128], BF16, tag="imgt")
                nc.sync.dma_start(out=img_t[:], in_=img[cam, :, ts(pt, 128)])
                kv_ps = psum.tile([128, KVROW], F32, tag="mm")
                nc.tensor.matmul(out=kv_ps[:], lhsT=img_t[:], rhs=wkvT_sb[:],
                                 start=True, stop=True)
                kv_bf = temps.tile([128, KVROW], BF16, tag="kvbf")
                nc.scalar.activation(out=kv_bf[:], in_=kv_ps[:], func=ACTF.Copy)
                nc.sync.dma_start(out=kv_dram[cam, ts(pt, 128), 0:KVROW], in_=kv_bf[:])
                if pt == 0:
                    nc.sync.dma_start(out=kv_dram[cam, 0:128 - IW, KVROW:2 * KVROW],
                                      in_=kv_bf[IW:128, :])
                else:
                    nc.sync.dma_start(
                        out=kv_dram[cam, pt * 128 - IW:pt * 128 - IW + 128,
                                    KVROW:2 * KVROW],
                        in_=kv_bf[:])

        # ------------- S2: queries, offsets -------------
        xyz1_sb = coords.tile([4, QPC], F32)
        nc.sync.dma_start(out=xyz1_sb[:2, :], in_=wxy_l)
        nc.sync.dma_start(out=xyz1_sb[2:4, :], in_=cst01)

        # q projection: q_sb[q, ch] per q-tile; bf16 copy and cq = q . bk
        q_sb = coords.tile([128, QPC], F32)        # [q-part, (qt,ch)]
        qbf_sb = singles.tile([128, QPC], BF16)
        cq_sb = singles.tile([128, NQT * HEADS], F32)
        for qt in range(NQT):
            q_ps = psum.tile([128, INNER], F32, tag="mm")
            nc.tensor.matmul(out=q_ps[:], lhsT=bev_sb[:, ts(qt, 128)],
                             rhs=wqT_sb[:], start=True, stop=True)
            nc.vector.tensor_tensor(out=q_sb[:, ts(qt, INNER)], in0=q_ps[:],
                                    in1=bq_sb[:], op=ALU.add)
            nc.scalar.activation(out=qbf_sb[:, ts(qt, INNER)],
                                 in_=q_sb[:, ts(qt, INNER)], func=ACTF.Copy)
            qbk = stats.tile([128, INNER], F32, tag="qbk")
            nc.vector.tensor_tensor(out=qbk[:], in0=q_sb[:, ts(qt, INNER)],
                                    in1=bk_sb[:], op=ALU.mult)
            nc.vector.tensor_reduce(
                out=cq_sb[:, ts(qt, HEADS)],
                in_=bass.AP(tensor=qbk.tensor, offset=qbk[:].offset,
                            ap=[qbk[:].ap[0], [DH, HEADS], [1, DH]]),
                axis=AX.X, op=ALU.add)

        # offsets: o1 = relu(w1 @ bev + b1); off = w2 @ o1 + b2  [16, QPC]
        o1_sb = coords.tile([DIM, QPC], BF16)
        for hf in range(2):
            o1_ps = psum2.tile([DIM, QPC // 2], F32, tag="wide")
            nc.tensor.matmul(out=o1_ps[:], lhsT=w1T_sb[:],
                             rhs=bev_sb[:, ts(hf, QPC // 2)], start=True, stop=True)
            nc.scalar.activation(out=o1_sb[:, ts(hf, QPC // 2)], in_=o1_ps[:],
                                 func=ACTF.Relu, bias=b1_sb[:], scale=1.0)
        off_sb = coords.tile([2 * NP, QPC], F32)   # rows: c*8+p (x 0-7, y 8-15)
        for hf in range(2):
            off_ps = psum2.tile([2 * NP, QPC // 2], F32, tag="wide")
            nc.tensor.matmul(out=off_ps[:], lhsT=w2T_sb[:],
                             rhs=o1_sb[:, ts(hf, QPC // 2)], start=True, stop=True)
            nc.scalar.activation(out=off_sb[:, ts(hf, QPC // 2)], in_=off_ps[:],
                                 func=ACTF.Identity, bias=b2_sb[:], scale=1.0)

        # transpose offsets once per q-tile: off_t_all [128, (qt, 16)]
        off_t_all = singles.tile([128, NQT * 16], F32)
        for qt in range(NQT):
            ot_ps = psum.tile([128, 2 * NP], F32, tag="mm")
            nc.tensor.transpose(out=ot_ps[:], in_=off_sb[:, ts(qt, 128)],
                                identity=ident[:2 * NP, :2 * NP])
            nc.scalar.activation(out=off_t_all[:, ts(qt, 2 * NP)], in_=ot_ps[:],
                                 func=ACTF.Copy)

        # ------------- S3: batched projection / sample coords -------------
        # pix per cam via folded M = K @ E[:3,:]; transpose into pxt_all.
        # pxt_all[q, (cam*8+qt)*3 + c], c in {u, v, z}
        pxt_all = coords.tile([128, NCQ * 3], F32)
        for cam in range(NCAM):
            pix_sb = coords.tile([3, QPC], F32, tag="pix")
            for hf in range(2):
                pix_ps = psum2.tile([3, QPC // 2], F32, tag="wide")
                nc.tensor.matmul(out=pix_ps[:],
                                 lhsT=MT_sb[:, ts(cam, 3)],
                                 rhs=xyz1_sb[:, ts(hf, QPC // 2)],
                                 start=True, stop=True)
                nc.scalar.activation(out=pix_sb[:, ts(hf, QPC // 2)], in_=pix_ps[:],
                                     func=ACTF.Copy)
            for qt in range(NQT):
                pt_ps = psum.tile([128, 3], F32, tag="mm")
                nc.tensor.transpose(out=pt_ps[:], in_=pix_sb[:, ts(qt, 128)],
                                    identity=ident[:3, :3])
                k = cam * NQT + qt
                nc.scalar.activation(out=pxt_all[:, k * 3:k * 3 + 3], in_=pt_ps[:],
                                     func=ACTF.Copy)

        pall = pxt_all[:]

        def pview(c0, n, inner=None):
            # view of pxt_all columns k*3 + c0 (k = 0..47); optionally a
            # trailing [1, inner] dim for consecutive channels
            apl = [pall.ap[0], [3, NCQ]] + ([[1, inner]] if inner else [])
            return bass.AP(tensor=pxt_all.tensor, offset=pall.offset + c0, ap=apl)

        # rden = 1 / max(z, 1e-6)   [128, 48]
        rden = coords.tile([128, NCQ], F32)
        nc.vector.tensor_scalar(out=rden[:], in0=pview(2, NCQ), scalar1=1e-6,
                                scalar2=None, op0=ALU.max)
        nc.vector.reciprocal(out=rden[:], in_=rden[:])
        # g = uv * rden, scaled to [-1,1]   [128, 96] cols k*2+c
        gxy = coords.tile([128, NCQ * 2], F32)
        rd = rden[:]
        nc.vector.tensor_tensor(
            out=gxy[:], in0=pview(0, NCQ, 2),
            in1=bass.AP(tensor=rden.tensor, offset=rd.offset,
                        ap=[rd.ap[0], [1, NCQ], [0, 2]]),
            op=ALU.mult)
        g = gxy[:]
        gx_view = bass.AP(tensor=gxy.tensor, offset=g.offset,
                          ap=[g.ap[0], [2, NCQ]])
        gy_view = bass.AP(tensor=gxy.tensor, offset=g.offset + 1,
                          ap=[g.ap[0], [2, NCQ]])
        nc.vector.tensor_scalar(out=gx_view, in0=gx_view,
                                scalar1=2.0 / (IW - 1), scalar2=1.0,
                                op0=ALU.mult, op1=ALU.subtract)
        nc.vector.tensor_scalar(out=gy_view, in0=gy_view,
                                scalar1=2.0 / (IH - 1), scalar2=1.0,
                                op0=ALU.mult, op1=ALU.subtract)

        # sxy = clip(off + g, -1, 1) -> pixel coords  [128, 768]
        # col layout: k*16 + c*8 + p  (k = cam*8+qt, c: 0=x 1=y)
        sxy = coords.tile([128, NCQ * 16], F32)
        sx = sxy[:]

        def sview(c0):
            return bass.AP(tensor=sxy.tensor, offset=sx.offset + c0 * NP,
                           ap=[sx.ap[0], [16, NCQ], [1, NP]])
        ot = off_t_all[:]
        for c in range(2):
            off_view = bass.AP(tensor=off_t_all.tensor,
                               offset=ot.offset + c * NP,
                               ap=[ot.ap[0], [0, NCAM], [16, NQT], [1, NP]])
            g_view = bass.AP(tensor=gxy.tensor, offset=g.offset + c,
                             ap=[g.ap[0], [2, NCQ], [0, NP]])
            nc.vector.tensor_tensor(out=sview(c), in0=off_view, in1=g_view,
                                    op=ALU.add)
        nc.vector.tensor_scalar(out=sxy[:], in0=sxy[:], scalar1=1.0,
                                scalar2=-1.0, op0=ALU.min, op1=ALU.max)
        nc.vector.tensor_scalar(out=sview(0), in0=sview(0), scalar1=1.0,
                                scalar2=0.5 * (IW - 1), op0=ALU.add, op1=ALU.mult)
        nc.vector.tensor_scalar(out=sview(1), in0=sview(1), scalar1=1.0,
                                scalar2=0.5 * (IH - 1), op0=ALU.add, op1=ALU.mult)

        # floor via +2^23 round-to-nearest, fixup so frac >= 0
        BIGF = 8388608.0
        rnd = coords.tile([128, NCQ * 16], F32)
        nc.vector.tensor_scalar(out=rnd[:], in0=sxy[:], scalar1=BIGF,
                                scalar2=BIGF, op0=ALU.add, op1=ALU.subtract)
        dfr = coords.tile([128, NCQ * 16], F32)
        nc.vector.tensor_tensor(out=dfr[:], in0=sxy[:], in1=rnd[:],
                                op=ALU.subtract)
        msk = coords.tile([128, NCQ * 16], F32)
        nc.vector.tensor_scalar(out=msk[:], in0=dfr[:], scalar1=0.0,
                                scalar2=None, op0=ALU.is_lt)
        x0y0 = coords.tile([128, NCQ * 16], F32)
        nc.vector.tensor_tensor(out=x0y0[:], in0=rnd[:], in1=msk[:],
                                op=ALU.subtract)
        # blend weights: wB = frac, wA = 1 - frac  [128, 768] resident
        wB_sb = singles.tile([128, NCQ * 16], F32)
        nc.vector.tensor_tensor(out=wB_sb[:], in0=dfr[:], in1=msk[:], op=ALU.add)
        wA_sb = singles.tile([128, NCQ * 16], F32)
        nc.vector.tensor_scalar(out=wA_sb[:], in0=wB_sb[:], scalar1=-1.0,
                                scalar2=1.0, op0=ALU.mult, op1=ALU.add)

        # row index: idx2 = camrow + y0*IW + x0    [128, 384] cols k*8+p
        idx2_all = singles.tile([128, NCQ * NP], F32)
        xy = x0y0[:]
        x_half = bass.AP(tensor=x0y0.tensor, offset=xy.offset,
                         ap=[xy.ap[0], [16, NCQ], [1, NP]])
        y_half = bass.AP(tensor=x0y0.tensor, offset=xy.offset + NP,
                         ap=[xy.ap[0], [16, NCQ], [1, NP]])
        nc.vector.tensor_scalar(out=idx2_all[:], in0=y_half, scalar1=float(IW),
                                scalar2=None, op0=ALU.mult)
        nc.vector.tensor_tensor(out=idx2_all[:], in0=idx2_all[:], in1=x_half,
                                op=ALU.add)
        for cam in range(1, NCAM):
            nc.vector.tensor_scalar(out=idx2_all[:, cam * NQT * NP:(cam + 1) * NQT * NP],
                                    in0=idx2_all[:, cam * NQT * NP:(cam + 1) * NQT * NP],
                                    scalar1=float(cam * PADROWS), scalar2=None,
                                    op0=ALU.add)

        # ------------- S4: gather index tables (16-wrapped) -------------
        # T_all[p, qt, cam*64 + r*8 + qh] = idx2_all[qh*16 + p%16, (cam,qt,r)]
        idxT = coords.tile([128, 3 * 128], F32)
        for i in range(3):
            tp = psum.tile([128, 128], F32, tag="mm")
            nc.tensor.transpose(out=tp[:], in_=idx2_all[:, ts(i, 128)],
                                identity=ident[:])
            nc.scalar.activation(out=idxT[:, ts(i, 128)], in_=tp[:], func=ACTF.Copy)
        T16f = coords.tile([16, NQT * 384], F32)
        t16 = T16f[:]
        for i in range(3):
            for qh in range(8):
                tb = psum.tile([16, 128], F32, tag="tb")
                nc.tensor.transpose(
                    out=tb[:], in_=idxT[:, i * 128 + qh * 16:i * 128 + qh * 16 + 16],
                    identity=ident[:])
                tbap = tb[:]
                src = bass.AP(tensor=tb.tensor, offset=tbap.offset,
                              ap=[tbap.ap[0], [64, 2], [8, NQT], [1, NP]])
                dst = bass.AP(tensor=T16f.tensor,
                              offset=t16.offset + (2 * i) * 64 + qh,
                              ap=[t16.ap[0], [64, 2], [384, NQT], [8, NP]])
                nc.scalar.activation(out=dst, in_=src, func=ACTF.Copy)
        T_all = singles.tile([128, NQT, 384], I16)
        for qt in range(NQT):
            rep_ps = psum2.tile([128, 384], F32, tag="wide")
            nc.tensor.matmul(out=rep_ps[:], lhsT=REP_sb[:],
                             rhs=T16f[:, qt * 384:(qt + 1) * 384],
                             start=True, stop=True)
            nc.vector.tensor_copy(out=T_all[:, qt, :], in_=rep_ps[:])

        # ------------- S5: main attention loop -------------
        NROWS = NCAM * PADROWS - 1
        for qt in range(NQT):
            wacc = stats.tile([128, INNER], F32, tag="wacc")
            nc.vector.memset(wacc[:], 0.0)
            for cam in range(NCAM):
                kvraw = gath.tile([128, NP, 4 * KVROW], BF16, tag="kvraw")
                kv_view = bass.AP(tensor=kv_dram.tensor, offset=0,
                                  ap=[[2 * KVROW, NROWS], [1, 4 * KVROW]])
                nc.gpsimd.dma_gather(
                    out_ap=kvraw[:], in_ap=kv_view,
                    idxs_ap=T_all[:, qt, ts(cam, 64)],
                    num_idxs=1024, num_idxs_reg=1024,
                    elem_size=4 * KVROW, elem_step=2 * KVROW,
                    single_packet=False)
                # x-blend: 8 rows of 512 [(y0,y1) x (K|V)]
                blkw = (cam * NQT + qt) * 16
                kvx = blend.tile([128, NP, 2 * KVROW], BF16, tag="kvx")
                for p in range(NP):
                    _lerp(nc, lerp_op, kvx[:, p, :],
                          kvraw[:, p, 0:2 * KVROW], kvraw[:, p, 2 * KVROW:4 * KVROW],
                          wA_sb[:, blkw + p:blkw + p + 1],
                          wB_sb[:, blkw + p:blkw + p + 1])
                # y-blend: 8 points of 256
                kvb = blend.tile([128, NP, KVROW], BF16, tag="kvb")
                for p in range(NP):
                    _lerp(nc, lerp_op, kvb[:, p, :],
                          kvx[:, p, 0:KVROW], kvx[:, p, KVROW:2 * KVROW],
                          wA_sb[:, blkw + 8 + p:blkw + 9 + p],
                          wB_sb[:, blkw + 8 + p:blkw + 9 + p])
                # K dot q -> sim [128, p, h]
                up = blend.tile([128, NP, INNER], BF16, tag="up")
                qv = qbf_sb[:, ts(qt, INNER)]
                nc.vector.tensor_tensor(
                    out=up[:], in0=kvb[:, :, 0:INNER],
                    in1=bass.AP(tensor=qbf_sb.tensor, offset=qv.offset,
                                ap=[qv.ap[0], [0, NP], [1, INNER]]),
                    op=ALU.mult)
                sim = stats.tile([128, NP, HEADS], F32, tag="sim")
                upap = up[:]
                nc.vector.tensor_reduce(
                    out=sim[:],
                    in_=bass.AP(tensor=up.tensor, offset=upap.offset,
                                ap=[upap.ap[0], [INNER, NP], [DH, HEADS], [1, DH]]),
                    axis=AX.X, op=ALU.add)
                cqv = cq_sb[:, ts(qt, HEADS)]
                nc.vector.tensor_tensor(
                    out=sim[:], in0=sim[:],
                    in1=bass.AP(tensor=cq_sb.tensor, offset=cqv.offset,
                                ap=[cqv.ap[0], [0, NP], [1, HEADS]]),
                    op=ALU.add)
                # softmax over p (and fold the 1/NCAM mean)
                esim = stats.tile([128, NP, HEADS], F32, tag="esim")
                nc.scalar.activation(out=esim[:], in_=sim[:], func=ACTF.Exp)
                ssum = stats.tile([128, HEADS], F32, tag="ssum")
                esap = esim[:]
                nc.vector.tensor_reduce(
                    out=ssum[:],
                    in_=bass.AP(tensor=esim.tensor, offset=esap.offset,
                                ap=[esap.ap[0], [1, HEADS], [HEADS, NP]]),
                    axis=AX.X, op=ALU.add)
                nc.vector.tensor_scalar(out=ssum[:], in0=ssum[:],
                                        scalar1=float(NCAM), scalar2=None,
                                        op0=ALU.mult)
                srec = stats.tile([128, HEADS], F32, tag="srec")
                nc.vector.reciprocal(out=srec[:], in_=ssum[:])
                att = stats.tile([128, NP, HEADS], BF16, tag="att")
                srap = srec[:]
                nc.vector.tensor_tensor(
                    out=att[:], in0=esim[:],
                    in1=bass.AP(tensor=srec.tensor, offset=srap.offset,
                                ap=[srap.ap[0], [0, NP], [1, HEADS]]),
                    op=ALU.mult)
                # weighted V sum over p
                vw = blend.tile([128, NP, INNER], BF16, tag="vw")
                atap = att[:]
                nc.vector.tensor_tensor(
                    out=vw[:], in0=kvb[:, :, INNER:KVROW],
                    in1=bass.AP(tensor=att.tensor, offset=atap.offset,
                                ap=[atap.ap[0], [HEADS, NP], [1, HEADS], [0, DH]]),
                    op=ALU.mult)
                wsum = stats.tile([128, INNER], F32, tag="wsum")
                vwap = vw[:]
                nc.vector.tensor_reduce(
                    out=wsum[:],
                    in_=bass.AP(tensor=vw.tensor, offset=vwap.offset,
                                ap=[vwap.ap[0], [1, INNER], [INNER, NP]]),
                    axis=AX.X, op=ALU.add)
                nc.vector.tensor_tensor(out=wacc[:], in0=wacc[:], in1=wsum[:],
                                        op=ALU.add)
            # + bv, then output projection
            nc.vector.tensor_tensor(out=wacc[:], in0=wacc[:], in1=bv_sb[:],
                                    op=ALU.add)
            wt_ps = psum.tile([128, 128], F32, tag="mm")
            nc.tensor.transpose(out=wt_ps[:], in_=wacc[:], identity=ident[:])
            waccT = temps.tile([128, 128], F32, tag="waccT")
            nc.scalar.activation(out=waccT[:], in_=wt_ps[:], func=ACTF.Copy)
            out_ps = psum.tile([128, DIM], F32, tag="mm")
            nc.tensor.matmul(out=out_ps[:], lhsT=waccT[:], rhs=wpT_sb[:],
                             start=True, stop=True)
            outf = temps.tile([128, DIM], F32, tag="outf")
            nc.vector.tensor_tensor(out=outf[:], in0=out_ps[:], in1=bp_sb[:],
                                    op=ALU.add)
            nc.sync.dma_start(out=out_l[ts(qt, 128), :], in_=outf[:])


# ---------------------------------------------------------------- host side
_CACHED = {}


def _build():
    if "nc" not in _CACHED:
        nc = bacc.Bacc("TRN2", target_bir_lowering=False, debug=False,
                       num_devices=NCORES)
        build_kernel(nc)
        nc.compile()
        _CACHED["nc"] = nc
    return _CACHED["nc"]


def make_in_maps(inputs):
    """Slice/transpose FULL inputs into 8 per-core input dicts (layout only)."""
    import ml_dtypes
    BF = ml_dtypes.bfloat16
    f = lambda x: np.ascontiguousarray(np.asarray(x, dtype=np.float32))
    fb = lambda x: np.ascontiguousarray(np.asarray(x, dtype=np.float32).astype(BF))
    bev = f(inputs["bev"]).reshape(B, DIM, HW)
    img_feats = f(inputs["img_feats"]).reshape(B, NCAM, DIM, IHW)
    Kc = f(inputs["K"])
    Ec = f(inputs["E"])
    world_xy = f(inputs["world_xy"]).reshape(2, HW)
    wq = f(inputs["wq"]); bq = f(inputs["bq"])
    wkv = f(inputs["wkv"]); bkv = f(inputs["bkv"])
    w_off1 = f(inputs["w_off1"]); b_off1 = f(inputs["b_off1"])
    w_off2 = f(inputs["w_off2"]); b_off2 = f(inputs["b_off2"])
    w_proj = f(inputs["w_proj"]); b_proj = f(inputs["b_proj"])

    # row-permute w_off2/b_off2 from (p, c) to (c, p) ordering
    perm = [p * 2 + c for c in range(2) for p in range(NP)]
    w2p = w_off2[perm, :]
    b2p = b_off2[perm]

    # folded projection M = K @ E[:3, :]  -> MT [4, NCAM*3] per batch
    Mfold = np.einsum('bnij,bnjk->bnik', Kc, Ec[:, :, :3, :])  # [B, NCAM, 3, 4]

    in_maps = []
    for core in range(NCORES):
        bc = core // (NCORES // B)
        q0 = (core % (NCORES // B)) * QPC
        m = {
            "img": fb(img_feats[bc]),
            "wkvT": fb(wkv.T),
            "bev_l": fb(bev[bc, :, q0:q0 + QPC]),
            "wxy_l": np.ascontiguousarray(world_xy[:, q0:q0 + QPC]),
            "MT": np.ascontiguousarray(
                Mfold[bc].transpose(2, 0, 1).reshape(4, NCAM * 3)),
            "wqT": fb(wq.T),
            "bq": bq.reshape(1, INNER),
            "w1T": fb(w_off1.T),
            "b1": b_off1.reshape(DIM, 1),
            "w2T": fb(w2p.T),
            "b2": b2p.reshape(2 * NP, 1),
            "wpT": np.ascontiguousarray(w_proj.T),
            "bp": b_proj.reshape(1, DIM),
            "bk": bkv[:INNER].reshape(1, INNER),
            "bv": bkv[INNER:].reshape(1, INNER),
            "cst01": np.concatenate([np.zeros((1, QPC), np.float32),
                                     np.ones((1, QPC), np.float32)], 0),
            "rep_in": (np.arange(128)[None, :] % 16 ==
                       np.arange(16)[:, None]).astype(np.float32),
        }
        in_maps.append(m)
    return in_maps


def assemble(results):
    """results: list of 8 dicts with out_l [QPC, DIM] -> [B, DIM, H, W]."""
    full = np.zeros((B, HW, DIM), dtype=np.float32)
    for core, r in enumerate(results):
        bc = core // (NCORES // B)
        q0 = (core % (NCORES // B)) * QPC
        full[bc, q0:q0 + QPC, :] = r["out_l"]
    return np.ascontiguousarray(full.transpose(0, 2, 1).reshape(B, DIM, H, W))


def kernel(**inputs):
    from concourse.bass_utils import run_bass_kernel_spmd
    nc = _build()
    in_maps = make_in_maps(inputs)
    res = run_bass_kernel_spmd(nc, in_maps, core_ids=list(range(NCORES)))
    return assemble(res.results)


if __name__ == "__main__":
    import reference
    inputs = {k: np.asarray(v) for k, v in reference.setup_inputs().items()}
    out = kernel(**inputs)
    exp = np.asarray(reference.reference(**{k: np.asarray(v) for k, v in inputs.items()}))
    err = np.abs(out - exp).max() / (np.abs(exp).max() + 1e-9)
    print("Relative error:", err)


# revision 11
# speedup vs baseline: 1.7135x; 1.4093x over previous
"""Trainium2 Bass kernel for CrossViewDeformableBlock (sparse deformable attention).

Contract: kernel(**inputs) -> np.ndarray takes FULL inputs (as from
setup_inputs()) and returns the FULL output [b, 128, 64, 64].

Sharding: 8 cores, q-parallel. Core c handles batch b_c = c//4 and query
range [(c%4)*1024, +1024) of the 64*64=4096 BEV queries. Each core builds
the bf16 K|V image tables for its 6 cameras on-device, computes projection
/ offsets / bilinear sample coordinates on-device (batched across all
(cam, q-tile) pairs), gathers 2-row pairs with dma_gather, blends corners
with a custom DVE lerp that runs in the 2X_1PORT perf mode, does the
point-softmax attention and output projection, and writes its [1024, 128]
output shard. The host only slices inputs, transposes weights (layout),
and concatenates shards.
"""

import math
import os
import numpy as np

import concourse.bass as bass
import concourse.mybir as mybir
import concourse.tile as tile
from concourse import bacc
from concourse.bass import ts
from concourse.masks import make_identity

# ---------------------------------------------------------------- constants
B, NCAM, H, W = 2, 6, 64, 64
HW = H * W                      # 4096 queries per batch
IH, IW = 32, 88                 # image feature h, w
IHW = IH * IW                   # 2816 positions
HEADS, DH, INNER = 4, 32, 128
NP = 8                          # sample points per query
DIM = 128
NCORES = 8
QPC = HW // (NCORES // B)       # 1024 queries per core
NQT = QPC // 128                # 8 q-tiles of 128
PADROWS = 2944                  # 23 * 128 rows in kv table (2816 + 128 pad)
KVROW = 2 * INNER               # 256 channels (K|V) per position
NCQ = NCAM * NQT                # 48 (cam, qtile) pairs
F32 = mybir.dt.float32
BF16 = mybir.dt.bfloat16
I16 = mybir.dt.int16
I32 = mybir.dt.int32

_USE_CUSTOM_LERP = True


def _register_lerp_op():
    """Register LERP2: out = in0*s0 + in1*s1 (per-partition scalars s0,s1).

    Registers both the 1x program (from lower()) and a hand-written
    2X_1PORT program so bf16 step-1 operands run at 2 elems/cycle when the
    emitted instruction sets perf_max.
    """
    from concourse.dve_spec import Spec, Src0, Src1, C0, C1, lower
    from concourse.dve_uop import (DveOpSpec, UopConfig, UopDpConfig, InpSel,
                                   AluInp, DelayInp, OutSel, OutPath, Trigger)
    from concourse.dve_spec import AluOp as DAlu
    from concourse.dve_ops import DveOp, OPS, _SUB_OPCODE_FOR_NAME, \
        _CUSTOM_DVE_ROW_BASE, _COMPILE_CACHE

    name = "LERP2"
    if name in _SUB_OPCODE_FOR_NAME:
        for op in OPS:
            if op.name == name:
                return op
    spec = Spec(
        body=Src0 * C0 + Src1 * C1,
        reference=lambda in0, in1, s0, s1, imm2: (
            in0.astype(np.float32) * s0 + in1.astype(np.float32) * s1
        ),
    )
    opcode = _CUSTOM_DVE_ROW_BASE + len(OPS)
    assert opcode < 0x20

    uops_1x = lower(spec, ver="v3")

    PD = DelayInp.PREV_DELAY
    PA = DelayInp.PREV_ALU_OUT
    A = AluInp
    MUL, ADD, BYP = DAlu.MULTIPLY, DAlu.ADD, DAlu.BYPASS

    def dp(op_, s0_, s1_, delay_sel, delay_en):
        return UopDpConfig(
            op=op_, alu_src0=s0_, alu_src1=s1_,
            delay=list(delay_sel), alu_out_enable=1, swap_enable=0,
            alu_out_a_enable=0, alu_out_b_enable=0,
            delay_enable=list(delay_en), idx0_sel=0, idx1_sel=0)

    # lanes: 0=SRC_0 1=SRC_1 2=SRC_0_HI 3=SRC_1_HI 4=CONST_0 5=CONST_1
    # stage-0 delay load: d_k <- lane k+1 (sel=PREV_DELAY)
    # => d0=SRC_1 d1=SRC_0_HI d2=SRC_1_HI d3=C0 d4=C1
    stages = [
        # m0l = SRC_0 (lane0) * C0 (d3)
        dp(MUL, A.PREV_ALU_OUT, A.PREV_DELAY_3,
           [PD, PD, PD, PD, PD, PA, PA], [1, 1, 1, 1, 1, 0, 0]),
        # m1l = SRC_1 (d0) * C1 (d4); d0 <- m0l
        dp(MUL, A.PREV_DELAY_0, A.PREV_DELAY_4,
           [PA, PD, PD, PD, PD, PA, PA], [1, 1, 1, 1, 1, 0, 0]),
        # lo = m0l (d0) + m1l (prev alu)
        dp(ADD, A.PREV_DELAY_0, A.PREV_ALU_OUT,
           [PA, PD, PD, PD, PD, PA, PA], [0, 1, 1, 1, 1, 0, 0]),
        # m0h = SRC_0_HI (d1) * C0 (d3); d0 <- lo
        dp(MUL, A.PREV_DELAY_1, A.PREV_DELAY_3,
           [PA, PA, PD, PD, PD, PA, PA], [1, 0, 1, 1, 1, 0, 0]),
        # m1h = SRC_1_HI (d2) * C1 (d4); d0 keep lo; d1 <- m0h
        dp(MUL, A.PREV_DELAY_2, A.PREV_DELAY_4,
           [PD, PA, PA, PA, PA, PA, PA], [1, 1, 0, 0, 0, 0, 0]),
        # hi = m0h (d1) + m1h (prev alu); d0 keep lo
        dp(ADD, A.PREV_DELAY_1, A.PREV_ALU_OUT,
           [PD, PA, PA, PA, PA, PA, PA], [1, 0, 0, 0, 0, 0, 0]),
        # out chain <- lo (d0); d0 <- hi
        dp(BYP, A.PREV_DELAY_0, A.PREV_ALU_OUT,
           [PA, PA, PA, PA, PA, PA, PA], [1, 0, 0, 0, 0, 0, 0]),
        # carry lo on alu chain; keep hi in d0
        dp(BYP, A.PREV_ALU_OUT, A.PREV_ALU_OUT,
           [PD, PA, PA, PA, PA, PA, PA], [1, 0, 0, 0, 0, 0, 0]),
    ]
    uop2x = UopConfig(
        inp=[InpSel.SRC_0, InpSel.SRC_1, InpSel.SRC_0_HI, InpSel.SRC_1_HI,
             InpSel.CONST_0, InpSel.CONST_1, InpSel.ZERO, InpSel.ZERO],
        inp_enable=[1, 1, 1, 1, 1, 1, 0, 0],
        out={OutPath.WR0_LO: OutSel.ALU_OUT, OutPath.WR0_HI: OutSel.DELAY_0,
             OutPath.WR1_LO: OutSel.ALU_OUT, OutPath.WR1_HI: OutSel.ALU_OUT},
        out_enable={OutPath.WR0_LO: 1, OutPath.WR0_HI: 1,
                    OutPath.WR1_LO: 0, OutPath.WR1_HI: 0},
        out_last_subdim_enable=0,
        force_two_data_zero=0, force_two_data_one=0,
        require_inp0=1, require_inp1=1, repeat_count=0,
        trigger=(Trigger.SRC_TENSOR_DONE, Trigger.NONE, Trigger.NONE),
        next_uop=(0, 0, 0),
        inc_parameter_index=0, enable_rev_ops=0, match_mask=0, valid_match=0,
        replace_on_match=0, clear_match=0, write_predicate_select=0,
        write_predicate_enable=0, delay_shift8=0, index_increment=0,
        index_clear=0, accum_enabled=0, v4={},
        datapath_config=stages,
    )
    full = DveOpSpec(name=name, opcode=opcode, uops=uops_1x,
                     uops_2x=[uop2x], rd1_en=True, perf_max=1)
    full.validate("v3")
    op = DveOp(name, spec, subdim=False, uops_sha={"v3": full.sha("v3")},
               perf_en={"v3": True})
    OPS.append(op)
    _SUB_OPCODE_FOR_NAME[name] = opcode
    from concourse import dve_ops as _do
    _do.CUSTOM_DVE_SPECS[name] = spec
    _do._COMPILE_CACHE[(name, "v3")] = full
    return op


def _lerp(nc, lerp_op, out, in0, in1, s0, s1):
    """out = in0*s0 + in1*s1 with s0/s1 [P,1] columns (bf16 data, 2x mode)."""
    if lerp_op is not None:
        inst = nc.vector._custom_dve(lerp_op, out=out, in0=in0, in1=in1,
                                     s0=s0, s1=s1)
        inst.ins.perf_max = 1
    else:
        nc.vector.tensor_tensor(out=out, in0=in1, in1=in0,
                                op=mybir.AluOpType.subtract)
        nc.vector.tensor_scalar(out=out, in0=out, scalar1=s1, scalar2=None,
                                op0=mybir.AluOpType.mult)
        nc.vector.tensor_tensor(out=out, in0=out, in1=in0,
                                op=mybir.AluOpType.add)


def build_kernel(nc):
    """Emit the SPMD program. All per-core variation comes via input data."""
    lerp_op = _register_lerp_op() if _USE_CUSTOM_LERP else None

    # ---------------- dram I/O ----------------
    img = nc.dram_tensor("img", [NCAM, DIM, IHW], F32, kind="ExternalInput").ap()
    wkvT = nc.dram_tensor("wkvT", [DIM, KVROW], F32, kind="ExternalInput").ap()
    bev_l = nc.dram_tensor("bev_l", [DIM, QPC], BF16, kind="ExternalInput").ap()
    wxy_l = nc.dram_tensor("wxy_l", [2, QPC], F32, kind="ExternalInput").ap()
    MT = nc.dram_tensor("MT", [4, NCAM * 3], F32, kind="ExternalInput").ap()
    wqT = nc.dram_tensor("wqT", [DIM, INNER], BF16, kind="ExternalInput").ap()
    bq = nc.dram_tensor("bq", [1, INNER], F32, kind="ExternalInput").ap()
    w1T = nc.dram_tensor("w1T", [DIM, DIM], BF16, kind="ExternalInput").ap()
    b1 = nc.dram_tensor("b1", [DIM, 1], F32, kind="ExternalInput").ap()
    w2T = nc.dram_tensor("w2T", [DIM, 2 * NP], BF16, kind="ExternalInput").ap()
    b2 = nc.dram_tensor("b2", [2 * NP, 1], F32, kind="ExternalInput").ap()
    wpT = nc.dram_tensor("wpT", [INNER, DIM], F32, kind="ExternalInput").ap()
    bp = nc.dram_tensor("bp", [1, DIM], F32, kind="ExternalInput").ap()
    bk = nc.dram_tensor("bk", [1, INNER], F32, kind="ExternalInput").ap()
    bv = nc.dram_tensor("bv", [1, INNER], F32, kind="ExternalInput").ap()
    cst01 = nc.dram_tensor("cst01", [2, QPC], F32, kind="ExternalInput").ap()
    rep_in = nc.dram_tensor("rep_in", [16, 128], F32, kind="ExternalInput").ap()
    out_l = nc.dram_tensor("out_l", [QPC, DIM], F32, kind="ExternalOutput").ap()

    with tile.TileContext(nc) as tc:
        _emit(tc, nc, lerp_op, img, wkvT, bev_l, wxy_l, MT, wqT, bq,
              w1T, b1, w2T, b2, wpT, bp, bk, bv, cst01, rep_in, out_l)
    return nc


def _emit(tc, nc, lerp_op, img, wkvT, bev_l, wxy_l, MT, wqT, bq,
          w1T, b1, w2T, b2, wpT, bp, bk, bv, cst01, rep_in, out_l):
    import contextlib
    ctx = contextlib.ExitStack()
    with ctx:
        singles = ctx.enter_context(tc.tile_pool(name="singles", bufs=1))
        temps = ctx.enter_context(tc.tile_pool(name="temps", bufs=3))
        gath = ctx.enter_context(tc.tile_pool(name="gath", bufs=4))
        coords = ctx.enter_context(tc.tile_pool(name="coords", bufs=1))
        blend = ctx.enter_context(tc.tile_pool(name="blend", bufs=3))
        stats = ctx.enter_context(tc.tile_pool(name="stats", bufs=6))
        psum = ctx.enter_context(tc.tile_pool(name="psum", bufs=2, space="PSUM"))
        psum2 = ctx.enter_context(tc.tile_pool(name="psum2", bufs=2, space="PSUM"))
        psumS1 = ctx.enter_context(tc.tile_pool(name="psumS1", bufs=2, space="PSUM"))
        dram = ctx.enter_context(tc.tile_pool(name="dram", bufs=1, space="DRAM"))

        AX = mybir.AxisListType
        ALU = mybir.AluOpType
        ACTF = mybir.ActivationFunctionType

        # ------------- resident tiles -------------
        ident = singles.tile([128, 128], F32)
        make_identity(nc, ident[:])
        wkvT_sb = singles.tile([DIM, KVROW], F32)
        nc.sync.dma_start(out=wkvT_sb[:], in_=wkvT)
        bev_sb = coords.tile([DIM, QPC], BF16)
        nc.sync.dma_start(out=bev_sb[:], in_=bev_l)
        wqT_sb = singles.tile([DIM, INNER], BF16)
        nc.sync.dma_start(out=wqT_sb[:], in_=wqT)
        w1T_sb = singles.tile([DIM, DIM], BF16)
        nc.sync.dma_start(out=w1T_sb[:], in_=w1T)
        w2T_sb = singles.tile([DIM, 2 * NP], BF16)
        nc.sync.dma_start(out=w2T_sb[:], in_=w2T)
        wpT_sb = singles.tile([INNER, DIM], F32)
        nc.sync.dma_start(out=wpT_sb[:], in_=wpT)
        b1_sb = singles.tile([DIM, 1], F32)
        nc.sync.dma_start(out=b1_sb[:], in_=b1)
        b2_sb = singles.tile([2 * NP, 1], F32)
        nc.sync.dma_start(out=b2_sb[:], in_=b2)
        ones_row = singles.tile([1, 128], F32)
        nc.vector.memset(ones_row[:], 1.0)

        def _rep128(name, src_ap, n):
            row = singles.tile([1, n], F32, tag=name + "_row")
            nc.sync.dma_start(out=row[:], in_=src_ap)
            ps = psum.tile([128, n], F32, tag="mm")
            nc.tensor.matmul(out=ps[:], lhsT=ones_row[:], rhs=row[:],
                             start=True, stop=True)
            t = singles.tile([128, n], F32, tag=name)
            nc.scalar.activation(out=t[:], in_=ps[:], func=ACTF.Copy)
            return t
        bq_sb = _rep128("bq128", bq, INNER)
        bp_sb = _rep128("bp128", bp, DIM)
        bk_sb = _rep128("bk128", bk, INNER)
        bv_sb = _rep128("bv128", bv, INNER)
        MT_sb = singles.tile([4, NCAM * 3], F32)
        nc.sync.dma_start(out=MT_sb[:], in_=MT)

        # DRAM scratch: paired-row kv table; row y*IW+x holds KV(y,x) ++ KV(y+1,x)
        kv_dram = dram.tile([NCAM, PADROWS, 2 * KVROW], BF16)
        REP_sb = singles.tile([16, 128], F32)
        nc.sync.dma_start(out=REP_sb[:], in_=rep_in)

        # ------------- S2: queries, offsets -------------
        xyz1_sb = coords.tile([4, QPC], F32)
        nc.sync.dma_start(out=xyz1_sb[:2, :], in_=wxy_l)
        nc.sync.dma_start(out=xyz1_sb[2:4, :], in_=cst01)

        # q projection: q_sb[q, ch] per q-tile; bf16 copy and cq = q . bk
        q_sb = coords.tile([128, QPC], F32)        # [q-part, (qt,ch)]
        qbf_sb = singles.tile([128, QPC], BF16)
        cq_sb = singles.tile([128, NQT * HEADS], F32)
        for qt in range(NQT):
            q_ps = psum.tile([128, INNER], F32, tag="mm")
            nc.tensor.matmul(out=q_ps[:], lhsT=bev_sb[:, ts(qt, 128)],
                             rhs=wqT_sb[:], start=True, stop=True)
            nc.vector.tensor_tensor(out=q_sb[:, ts(qt, INNER)], in0=q_ps[:],
                                    in1=bq_sb[:], op=ALU.add)
            nc.scalar.activation(out=qbf_sb[:, ts(qt, INNER)],
                                 in_=q_sb[:, ts(qt, INNER)], func=ACTF.Copy)
            qbk = stats.tile([128, INNER], F32, tag="qbk")
            nc.vector.tensor_tensor(out=qbk[:], in0=q_sb[:, ts(qt, INNER)],
                                    in1=bk_sb[:], op=ALU.mult)
            nc.vector.tensor_reduce(
                out=cq_sb[:, ts(qt, HEADS)],
                in_=bass.AP(tensor=qbk.tensor, offset=qbk[:].offset,
                            ap=[qbk[:].ap[0], [DH, HEADS], [1, DH]]),
                axis=AX.X, op=ALU.add)

        # offsets: o1 = relu(w1 @ bev + b1); off = w2 @ o1 + b2  [16, QPC]
        o1_sb = coords.tile([DIM, QPC], BF16)
        for hf in range(2):
            o1_ps = psum2.tile([DIM, QPC // 2], F32, tag="wide")
            nc.tensor.matmul(out=o1_ps[:], lhsT=w1T_sb[:],
                             rhs=bev_sb[:, ts(hf, QPC // 2)], start=True, stop=True)
            nc.scalar.activation(out=o1_sb[:, ts(hf, QPC // 2)], in_=o1_ps[:],
                                 func=ACTF.Relu, bias=b1_sb[:], scale=1.0)
        off_sb = coords.tile([2 * NP, QPC], F32)   # rows: c*8+p (x 0-7, y 8-15)
        for hf in range(2):
            off_ps = psum2.tile([2 * NP, QPC // 2], F32, tag="wide")
            nc.tensor.matmul(out=off_ps[:], lhsT=w2T_sb[:],
                             rhs=o1_sb[:, ts(hf, QPC // 2)], start=True, stop=True)
            nc.scalar.activation(out=off_sb[:, ts(hf, QPC // 2)], in_=off_ps[:],
                                 func=ACTF.Identity, bias=b2_sb[:], scale=1.0)

        # transpose offsets once per q-tile: off_t_all [128, (qt, 16)]
        off_t_all = singles.tile([128, NQT * 16], F32)
        for qt in range(NQT):
            ot_ps = psum.tile([128, 2 * NP], F32, tag="mm")
            nc.tensor.transpose(out=ot_ps[:], in_=off_sb[:, ts(qt, 128)],
                                identity=ident[:2 * NP, :2 * NP])
            nc.scalar.activation(out=off_t_all[:, ts(qt, 2 * NP)], in_=ot_ps[:],
                                 func=ACTF.Copy)

        # ------------- S3: batched projection / sample coords -------------
        # pix per cam via folded M = K @ E[:3,:]; transpose into pxt_all.
        # pxt_all[q, (cam*8+qt)*3 + c], c in {u, v, z}
        pxt_all = coords.tile([128, NCQ * 3], F32)
        for cam in range(NCAM):
            pix_sb = coords.tile([3, QPC], F32, tag="pix")
            for hf in range(2):
                pix_ps = psum2.tile([3, QPC // 2], F32, tag="wide")
                nc.tensor.matmul(out=pix_ps[:],
                                 lhsT=MT_sb[:, ts(cam, 3)],
                                 rhs=xyz1_sb[:, ts(hf, QPC // 2)],
                                 start=True, stop=True)
                nc.scalar.activation(out=pix_sb[:, ts(hf, QPC // 2)], in_=pix_ps[:],
                                     func=ACTF.Copy)
            for qt in range(NQT):
                pt_ps = psum.tile([128, 3], F32, tag="mm")
                nc.tensor.transpose(out=pt_ps[:], in_=pix_sb[:, ts(qt, 128)],
                                    identity=ident[:3, :3])
                k = cam * NQT + qt
                nc.scalar.activation(out=pxt_all[:, k * 3:k * 3 + 3], in_=pt_ps[:],
                                     func=ACTF.Copy)

        pall = pxt_all[:]

        def pview(c0, n, inner=None):
            # view of pxt_all columns k*3 + c0 (k = 0..47); optionally a
            # trailing [1, inner] dim for consecutive channels
            apl = [pall.ap[0], [3, NCQ]] + ([[1, inner]] if inner else [])
            return bass.AP(tensor=pxt_all.tensor, offset=pall.offset + c0, ap=apl)

        # rden = 1 / max(z, 1e-6)   [128, 48]
        rden = coords.tile([128, NCQ], F32)
        nc.vector.tensor_scalar(out=rden[:], in0=pview(2, NCQ), scalar1=1e-6,
                                scalar2=None, op0=ALU.max)
        nc.vector.reciprocal(out=rden[:], in_=rden[:])
        # g = uv * rden, scaled to [-1,1]   [128, 96] cols k*2+c
        gxy = coords.tile([128, NCQ * 2], F32)
        rd = rden[:]
        nc.vector.tensor_tensor(
            out=gxy[:], in0=pview(0, NCQ, 2),
            in1=bass.AP(tensor=rden.tensor, offset=rd.offset,
                        ap=[rd.ap[0], [1, NCQ], [0, 2]]),
            op=ALU.mult)
        g = gxy[:]
        gx_view = bass.AP(tensor=gxy.tensor, offset=g.offset,
                          ap=[g.ap[0], [2, NCQ]])
        gy_view = bass.AP(tensor=gxy.tensor, offset=g.offset + 1,
                          ap=[g.ap[0], [2, NCQ]])
        nc.vector.tensor_scalar(out=gx_view, in0=gx_view,
                                scalar1=2.0 / (IW - 1), scalar2=1.0,
                                op0=ALU.mult, op1=ALU.subtract)
        nc.vector.tensor_scalar(out=gy_view, in0=gy_view,
                                scalar1=2.0 / (IH - 1), scalar2=1.0,
                                op0=ALU.mult, op1=ALU.subtract)

        # sxy = clip(off + g, -1, 1) -> pixel coords  [128, 768]
        # col layout: k*16 + c*8 + p  (k = cam*8+qt, c: 0=x 1=y)
        sxy = coords.tile([128, NCQ * 16], F32)
        sx = sxy[:]

        def sview(c0):
            return bass.AP(tensor=sxy.tensor, offset=sx.offset + c0 * NP,
                           ap=[sx.ap[0], [16, NCQ], [1, NP]])
        ot = off_t_all[:]
        for c in range(2):
            off_view = bass.AP(tensor=off_t_all.tensor,
                               offset=ot.offset + c * NP,
                               ap=[ot.ap[0], [0, NCAM], [16, NQT], [1, NP]])
            g_view = bass.AP(tensor=gxy.tensor, offset=g.offset + c,
                             ap=[g.ap[0], [2, NCQ], [0, NP]])
            nc.vector.tensor_tensor(out=sview(c), in0=off_view, in1=g_view,
                                    op=ALU.add)
        nc.vector.tensor_scalar(out=sxy[:], in0=sxy[:], scalar1=1.0,
                                scalar2=-1.0, op0=ALU.min, op1=ALU.max)
        nc.vector.tensor_scalar(out=sview(0), in0=sview(0), scalar1=1.0,
                                scalar2=0.5 * (IW - 1), op0=ALU.add, op1=ALU.mult)
        nc.vector.tensor_scalar(out=sview(1), in0=sview(1), scalar1=1.0,
                                scalar2=0.5 * (IH - 1), op0=ALU.add, op1=ALU.mult)

        # floor via +2^23 round-to-nearest, fixup so frac >= 0
        BIGF = 8388608.0
        rnd = coords.tile([128, NCQ * 16], F32)
        nc.vector.tensor_scalar(out=rnd[:], in0=sxy[:], scalar1=BIGF,
                                scalar2=BIGF, op0=ALU.add, op1=ALU.subtract)
        dfr = coords.tile([128, NCQ * 16], F32)
        nc.vector.tensor_tensor(out=dfr[:], in0=sxy[:], in1=rnd[:],
                                op=ALU.subtract)
        msk = coords.tile([128, NCQ * 16], F32)
        nc.vector.tensor_scalar(out=msk[:], in0=dfr[:], scalar1=0.0,
                                scalar2=None, op0=ALU.is_lt)
        x0y0 = coords.tile([128, NCQ * 16], F32)
        nc.vector.tensor_tensor(out=x0y0[:], in0=rnd[:], in1=msk[:],
                                op=ALU.subtract)
        # blend weights: wB = frac, wA = 1 - frac  [128, 768] resident
        wB_sb = singles.tile([128, NCQ * 16], F32)
        nc.vector.tensor_tensor(out=wB_sb[:], in0=dfr[:], in1=msk[:], op=ALU.add)
        wA_sb = singles.tile([128, NCQ * 16], F32)
        nc.vector.tensor_scalar(out=wA_sb[:], in0=wB_sb[:], scalar1=-1.0,
                                scalar2=1.0, op0=ALU.mult, op1=ALU.add)

        # row index: idx2 = camrow + y0*IW + x0    [128, 384] cols k*8+p
        idx2_all = singles.tile([128, NCQ * NP], F32)
        xy = x0y0[:]
        x_half = bass.AP(tensor=x0y0.tensor, offset=xy.offset,
                         ap=[xy.ap[0], [16, NCQ], [1, NP]])
        y_half = bass.AP(tensor=x0y0.tensor, offset=xy.offset + NP,
                         ap=[xy.ap[0], [16, NCQ], [1, NP]])
        nc.vector.tensor_scalar(out=idx2_all[:], in0=y_half, scalar1=float(IW),
                                scalar2=None, op0=ALU.mult)
        nc.vector.tensor_tensor(out=idx2_all[:], in0=idx2_all[:], in1=x_half,
                                op=ALU.add)

        # ------------- S4: gather index tables (16-wrapped) -------------
        # T_all[p, qt, cam*64 + r*8 + qh] = idx2_all[qh*16 + p%16, (cam,qt,r)]
        idxT = coords.tile([128, 3 * 128], F32)
        for i in range(3):
            tp = psum.tile([128, 128], F32, tag="mm")
            nc.tensor.transpose(out=tp[:], in_=idx2_all[:, ts(i, 128)],
                                identity=ident[:])
            nc.scalar.activation(out=idxT[:, ts(i, 128)], in_=tp[:], func=ACTF.Copy)
        T16f = coords.tile([16, NQT * 384], F32)
        t16 = T16f[:]
        for i in range(3):
            for qh in range(8):
                tb = psum.tile([16, 128], F32, tag="tb")
                nc.tensor.transpose(
                    out=tb[:], in_=idxT[:, i * 128 + qh * 16:i * 128 + qh * 16 + 16],
                    identity=ident[:])
                tbap = tb[:]
                src = bass.AP(tensor=tb.tensor, offset=tbap.offset,
                              ap=[tbap.ap[0], [64, 2], [8, NQT], [1, NP]])
                dst = bass.AP(tensor=T16f.tensor,
                              offset=t16.offset + (2 * i) * 64 + qh,
                              ap=[t16.ap[0], [64, 2], [384, NQT], [8, NP]])
                nc.scalar.activation(out=dst, in_=src, func=ACTF.Copy)
        T_all = singles.tile([128, NQT, 384], I16)
        for qt in range(NQT):
            rep_ps = psum2.tile([128, 384], F32, tag="wide")
            nc.tensor.matmul(out=rep_ps[:], lhsT=REP_sb[:],
                             rhs=T16f[:, qt * 384:(qt + 1) * 384],
                             start=True, stop=True)
            nc.vector.tensor_copy(out=T_all[:, qt, :], in_=rep_ps[:])

        # ------------- S1: kv tables (interleaved with main loop) -------------
        zt = singles.tile([128, KVROW], BF16)
        nc.vector.memset(zt[:], 0)
        NPT = IHW // 128  # 22 position tiles

        def build_cam_table(cam):
            nc.sync.dma_start(out=kv_dram[cam, IHW:PADROWS, 0:KVROW], in_=zt[:])
            nc.sync.dma_start(out=kv_dram[cam, IHW:PADROWS, KVROW:2 * KVROW],
                              in_=zt[:])
            for pt in range(NPT):
                img_t = temps.tile([128, 128], F32, tag="imgt")
                nc.sync.dma_start(out=img_t[:], in_=img[cam, :, ts(pt, 128)])
                kv_ps = psumS1.tile([128, KVROW], F32, tag="kvps")
                nc.tensor.matmul(out=kv_ps[:], lhsT=img_t[:], rhs=wkvT_sb[:],
                                 start=True, stop=True)
                kv_bf = temps.tile([128, KVROW], BF16, tag="kvbf")
                nc.scalar.activation(out=kv_bf[:], in_=kv_ps[:], func=ACTF.Copy)
                nc.sync.dma_start(out=kv_dram[cam, ts(pt, 128), 0:KVROW], in_=kv_bf[:])
            # second half: row r col 256:512 = KV(r+IW), via one shifted D2D
            nc.sync.dma_start(out=kv_dram[cam, 0:IHW, KVROW:2 * KVROW],
                              in_=kv_dram[cam, IW:IHW + IW, 0:KVROW])

        # ------------- S5: main attention loop (cam outer) -------------
        wacc_all = singles.tile([128, NQT, INNER], F32)
        nc.vector.memset(wacc_all[:], 0.0)
        for cam in range(NCAM):
            build_cam_table(cam)
            for qt in range(NQT):
                wacc = wacc_all[:, qt, :]
                kvraw = gath.tile([128, NP, 4 * KVROW], BF16, tag="kvraw")
                kv_view = bass.AP(tensor=kv_dram.tensor,
                                  offset=cam * PADROWS * 2 * KVROW,
                                  ap=[[2 * KVROW, PADROWS - 1], [1, 4 * KVROW]])
                nc.gpsimd.dma_gather(
                    out_ap=kvraw[:], in_ap=kv_view,
                    idxs_ap=T_all[:, qt, ts(cam, 64)],
                    num_idxs=1024, num_idxs_reg=1024,
                    elem_size=4 * KVROW, elem_step=2 * KVROW,
                    single_packet=False)
                # x-blend: 8 rows of 512 [(y0,y1) x (K|V)]
                blkw = (cam * NQT + qt) * 16
                kvx = blend.tile([128, NP, 2 * KVROW], BF16, tag="kvx")
                for p in range(NP):
                    _lerp(nc, lerp_op, kvx[:, p, :],
                          kvraw[:, p, 0:2 * KVROW], kvraw[:, p, 2 * KVROW:4 * KVROW],
                          wA_sb[:, blkw + p:blkw + p + 1],
                          wB_sb[:, blkw + p:blkw + p + 1])
                # y-blend: 8 points of 256
                kvb = blend.tile([128, NP, KVROW], BF16, tag="kvb")
                for p in range(NP):
                    _lerp(nc, lerp_op, kvb[:, p, :],
                          kvx[:, p, 0:KVROW], kvx[:, p, KVROW:2 * KVROW],
                          wA_sb[:, blkw + 8 + p:blkw + 9 + p],
                          wB_sb[:, blkw + 8 + p:blkw + 9 + p])
                # K dot q -> sim [128, p, h]
                up = blend.tile([128, NP, INNER], BF16, tag="up")
                qv = qbf_sb[:, ts(qt, INNER)]
                nc.vector.tensor_tensor(
                    out=up[:], in0=kvb[:, :, 0:INNER],
                    in1=bass.AP(tensor=qbf_sb.tensor, offset=qv.offset,
                                ap=[qv.ap[0], [0, NP], [1, INNER]]),
                    op=ALU.mult)
                sim = stats.tile([128, NP, HEADS], F32, tag="sim")
                upap = up[:]
                nc.vector.tensor_reduce(
                    out=sim[:],
                    in_=bass.AP(tensor=up.tensor, offset=upap.offset,
                                ap=[upap.ap[0], [INNER, NP], [DH, HEADS], [1, DH]]),
                    axis=AX.X, op=ALU.add)
                cqv = cq_sb[:, ts(qt, HEADS)]
                nc.vector.tensor_tensor(
                    out=sim[:], in0=sim[:],
                    in1=bass.AP(tensor=cq_sb.tensor, offset=cqv.offset,
                                ap=[cqv.ap[0], [0, NP], [1, HEADS]]),
                    op=ALU.add)
                # softmax over p (and fold the 1/NCAM mean)
                esim = stats.tile([128, NP, HEADS], F32, tag="esim")
                nc.scalar.activation(out=esim[:], in_=sim[:], func=ACTF.Exp)
                ssum = stats.tile([128, HEADS], F32, tag="ssum")
                esap = esim[:]
                nc.vector.tensor_reduce(
                    out=ssum[:],
                    in_=bass.AP(tensor=esim.tensor, offset=esap.offset,
                                ap=[esap.ap[0], [1, HEADS], [HEADS, NP]]),
                    axis=AX.X, op=ALU.add)
                nc.vector.tensor_scalar(out=ssum[:], in0=ssum[:],
                                        scalar1=float(NCAM), scalar2=None,
                                        op0=ALU.mult)
                srec = stats.tile([128, HEADS], F32, tag="srec")
                nc.vector.reciprocal(out=srec[:], in_=ssum[:])
                att = stats.tile([128, NP, HEADS], BF16, tag="att")
                srap = srec[:]
                nc.vector.tensor_tensor(
                    out=att[:], in0=esim[:],
                    in1=bass.AP(tensor=srec.tensor, offset=srap.offset,
                                ap=[srap.ap[0], [0, NP], [1, HEADS]]),
                    op=ALU.mult)
                # weighted V sum over p
                vw = blend.tile([128, NP, INNER], BF16, tag="vw")
                atap = att[:]
                nc.vector.tensor_tensor(
                    out=vw[:], in0=kvb[:, :, INNER:KVROW],
                    in1=bass.AP(tensor=att.tensor, offset=atap.offset,
                                ap=[atap.ap[0], [HEADS, NP], [1, HEADS], [0, DH]]),
                    op=ALU.mult)
                wsum = stats.tile([128, INNER], F32, tag="wsum")
                vwap = vw[:]
                nc.vector.tensor_reduce(
                    out=wsum[:],
                    in_=bass.AP(tensor=vw.tensor, offset=vwap.offset,
                                ap=[vwap.ap[0], [1, INNER], [INNER, NP]]),
                    axis=AX.X, op=ALU.add)
                nc.vector.tensor_tensor(out=wacc, in0=wacc, in1=wsum[:],
                                        op=ALU.add)

        # ------------- S6: output projection per q-tile -------------
        for qt in range(NQT):
            wacc = wacc_all[:, qt, :]
            nc.vector.tensor_tensor(out=wacc, in0=wacc, in1=bv_sb[:],
                                    op=ALU.add)
            wt_ps = psum.tile([128, 128], F32, tag="mm")
            nc.tensor.transpose(out=wt_ps[:], in_=wacc, identity=ident[:])
            waccT = temps.tile([128, 128], F32, tag="waccT")
            nc.scalar.activation(out=waccT[:], in_=wt_ps[:], func=ACTF.Copy)
            out_ps = psum.tile([128, DIM], F32, tag="mm")
            nc.tensor.matmul(out=out_ps[:], lhsT=waccT[:], rhs=wpT_sb[:],
                             start=True, stop=True)
            outf = temps.tile([128, DIM], F32, tag="outf")
            nc.vector.tensor_tensor(out=outf[:], in0=out_ps[:], in1=bp_sb[:],
                                    op=ALU.add)
            nc.sync.dma_start(out=out_l[ts(qt, 128), :], in_=outf[:])


# ---------------------------------------------------------------- host side
_CACHED = {}


def _build():
    if "nc" not in _CACHED:
        nc = bacc.Bacc("TRN2", target_bir_lowering=False, debug=False,
                       num_devices=NCORES)
        build_kernel(nc)
        nc.compile()
        _CACHED["nc"] = nc
    return _CACHED["nc"]


def make_in_maps(inputs):
    """Slice/transpose FULL inputs into 8 per-core input dicts (layout only)."""
    import ml_dtypes
    BF = ml_dtypes.bfloat16
    f = lambda x: np.ascontiguousarray(np.asarray(x, dtype=np.float32))
    fb = lambda x: np.ascontiguousarray(np.asarray(x, dtype=np.float32).astype(BF))
    bev = f(inputs["bev"]).reshape(B, DIM, HW)
    img_feats = f(inputs["img_feats"]).reshape(B, NCAM, DIM, IHW)
    Kc = f(inputs["K"])
    Ec = f(inputs["E"])
    world_xy = f(inputs["world_xy"]).reshape(2, HW)
    wq = f(inputs["wq"]); bq = f(inputs["bq"])
    wkv = f(inputs["wkv"]); bkv = f(inputs["bkv"])
    w_off1 = f(inputs["w_off1"]); b_off1 = f(inputs["b_off1"])
    w_off2 = f(inputs["w_off2"]); b_off2 = f(inputs["b_off2"])
    w_proj = f(inputs["w_proj"]); b_proj = f(inputs["b_proj"])

    # row-permute w_off2/b_off2 from (p, c) to (c, p) ordering
    perm = [p * 2 + c for c in range(2) for p in range(NP)]
    w2p = w_off2[perm, :]
    b2p = b_off2[perm]

    # folded projection M = K @ E[:3, :]  -> MT [4, NCAM*3] per batch
    Mfold = np.einsum('bnij,bnjk->bnik', Kc, Ec[:, :, :3, :])  # [B, NCAM, 3, 4]

    in_maps = []
    for core in range(NCORES):
        bc = core // (NCORES // B)
        q0 = (core % (NCORES // B)) * QPC
        m = {
            "img": np.ascontiguousarray(img_feats[bc]),
            "wkvT": np.ascontiguousarray(wkv.T),
            "bev_l": fb(bev[bc, :, q0:q0 + QPC]),
            "wxy_l": np.ascontiguousarray(world_xy[:, q0:q0 + QPC]),
            "MT": np.ascontiguousarray(
                Mfold[bc].transpose(2, 0, 1).reshape(4, NCAM * 3)),
            "wqT": fb(wq.T),
            "bq": bq.reshape(1, INNER),
            "w1T": fb(w_off1.T),
            "b1": b_off1.reshape(DIM, 1),
            "w2T": fb(w2p.T),
            "b2": b2p.reshape(2 * NP, 1),
            "wpT": np.ascontiguousarray(w_proj.T),
            "bp": b_proj.reshape(1, DIM),
            "bk": bkv[:INNER].reshape(1, INNER),
            "bv": bkv[INNER:].reshape(1, INNER),
            "cst01": np.concatenate([np.zeros((1, QPC), np.float32),
                                     np.ones((1, QPC), np.float32)], 0),
            "rep_in": (np.arange(128)[None, :] % 16 ==
                       np.arange(16)[:, None]).astype(np.float32),
        }
        in_maps.append(m)
    return in_maps


def assemble(results):
    """results: list of 8 dicts with out_l [QPC, DIM] -> [B, DIM, H, W]."""
    full = np.zeros((B, HW, DIM), dtype=np.float32)
    for core, r in enumerate(results):
        bc = core // (NCORES // B)
        q0 = (core % (NCORES // B)) * QPC
        full[bc, q0:q0 + QPC, :] = r["out_l"]
    return np.ascontiguousarray(full.transpose(0, 2, 1).reshape(B, DIM, H, W))


def kernel(**inputs):
    from concourse.bass_utils import run_bass_kernel_spmd
    nc = _build()
    in_maps = make_in_maps(inputs)
    res = run_bass_kernel_spmd(nc, in_maps, core_ids=list(range(NCORES)))
    return assemble(res.results)


if __name__ == "__main__":
    import reference
    inputs = {k: np.asarray(v) for k, v in reference.setup_inputs().items()}
    out = kernel(**inputs)
    exp = np.asarray(reference.reference(**{k: np.asarray(v) for k, v in inputs.items()}))
    err = np.abs(out - exp).max() / (np.abs(exp).max() + 1e-9)
    print("Relative error:", err)
